# revision 1
# baseline (speedup 1.0000x reference)
import sys
sys.path.insert(0, "/opt/trn_rl_repo")
import numpy as np
import concourse.bass as bass
import concourse.bacc as bacc
import concourse.mybir as mybir
import concourse.tile as tile
from concourse.bass_utils import run_bass_kernel_spmd

# Problem constants (hardcoded per contract)
N = 20000
T = 20
D = 64
H = 64
W = 3
NCORES = 8
NPAD = 24576            # 8 * 3072
PER_CORE = 3072         # padded per-core stocks
C = 512                 # chunk size
NPAIR = 3               # pairs of chunks per core (6 chunks)
dt = mybir.dt

_cache = {}


def _build_program():
    """Bass program: GRU over T steps for 3 weeks x 3 chunk-pairs of 1024 stocks.

    Layouts per (week, pair):
      xh_A/xh_B [128, 21*C]: rows 0:64 x features (slot t = x_t), rows 64:128
        h state (slot t = h_{t-1});  hs [128, 21*C]: packed h (A on 0:64,
        B on 64:128), slot t = h_{t-1}.
    Gate matmuls K=128 (stacked [x;h]) with M=64, col-tiled A->psum[0:64],
    B->psum[64:128] (tile_position=(0,64)).  fp32r for full-rate fp32.
    """
    nc = bacc.Bacc("TRN2", target_bir_lowering=False, debug=False,
                   num_devices=NCORES)
    SLOTS = 21 * C
    x_in = nc.declare_dram_parameter("x", [W, NPAIR, 2, 64, T * C], dt.float32,
                                     isOutput=False)
    wl_in = nc.declare_dram_parameter("wl", [128, W * 4 * 64], dt.float32,
                                      isOutput=False)
    bl_in = nc.declare_dram_parameter("bl", [128, W * 4], dt.float32,
                                      isOutput=False)
    hs_out = nc.declare_dram_parameter("hs", [W, NPAIR, 128, T * C], dt.float32,
                                       isOutput=True)
    f32r = dt.float32r
    AF = mybir.ActivationFunctionType
    OP = mybir.AluOpType

    with tile.TileContext(nc) as tc:
        with tc.tile_pool(name="wpool", bufs=1) as wpool, \
             tc.tile_pool(name="xh", bufs=1) as xhp, \
             tc.tile_pool(name="hsp", bufs=1) as hsp, \
             tc.tile_pool(name="gate", bufs=2) as gp, \
             tc.tile_pool(name="psum", bufs=2, space="PSUM") as pp:
            wl = wpool.tile([128, W * 4 * 64], dt.float32)
            bl = wpool.tile([128, W * 4], dt.float32)
            nc.sync.dma_start(out=wl[:], in_=wl_in[:, :])
            nc.sync.dma_start(out=bl[:], in_=bl_in[:, :])

            for w in range(W):
                for p in range(NPAIR):
                    xh_A = xhp.tile([128, SLOTS], dt.float32, tag="xha")
                    xh_B = xhp.tile([128, SLOTS], dt.float32, tag="xhb")
                    hs = hsp.tile([128, SLOTS], dt.float32, tag="hs")
                    nc.sync.dma_start(out=xh_A[0:64, 0:T * C], in_=x_in[w, p, 0])
                    nc.sync.dma_start(out=xh_B[0:64, 0:T * C], in_=x_in[w, p, 1])
                    nc.vector.memset(xh_A[64:128, 0:C], 0.0)
                    nc.vector.memset(xh_B[64:128, 0:C], 0.0)
                    nc.vector.memset(hs[:, 0:C], 0.0)
                    for t in range(T):
                        sl = slice(t * C, (t + 1) * C)
                        sl1 = slice((t + 1) * C, (t + 2) * C)
                        ps = []
                        for g in range(4):
                            pg = pp.tile([128, C], dt.float32, tag=f"g{g}")
                            lt = wl[:, (w * 4 + g) * 64:(w * 4 + g + 1) * 64]
                            nc.tensor.matmul(out=pg[0:64, :],
                                             lhsT=lt,
                                             rhs=xh_A[:, sl],
                                             start=True, stop=True)
                            nc.tensor.matmul(out=pg[64:128, :],
                                             lhsT=lt,
                                             rhs=xh_B[:, sl],
                                             start=True, stop=True,
                                             tile_position=(0, 64))
                            ps.append(pg)
                        r = gp.tile([128, C], dt.float32, tag="r")
                        z = gp.tile([128, C], dt.float32, tag="z")
                        v = gp.tile([128, C], dt.float32, tag="v")
                        wo = gp.tile([128, C], dt.float32, tag="wo")
                        c_ = gp.tile([128, C], dt.float32, tag="c")
                        s_ = gp.tile([128, C], dt.float32, tag="s")
                        t_ = gp.tile([128, C], dt.float32, tag="t")
                        nc.scalar.activation(out=r[:], in_=ps[0][:], func=AF.Sigmoid,
                                             bias=bl[:, (w * 4 + 0):(w * 4 + 1)])
                        nc.scalar.activation(out=z[:], in_=ps[1][:], func=AF.Sigmoid,
                                             bias=bl[:, (w * 4 + 1):(w * 4 + 2)])
                        # v = (hn + b_hn) * r ; wo = (xn + b_in) + v
                        nc.vector.scalar_tensor_tensor(
                            out=v[:], in0=ps[2][:], scalar=bl[:, (w * 4 + 2):(w * 4 + 3)],
                            in1=r[:], op0=OP.add, op1=OP.mult)
                        nc.vector.scalar_tensor_tensor(
                            out=wo[:], in0=ps[3][:], scalar=bl[:, (w * 4 + 3):(w * 4 + 4)],
                            in1=v[:], op0=OP.add, op1=OP.add)
                        nc.scalar.activation(out=c_[:], in_=wo[:], func=AF.Tanh)
                        nc.vector.tensor_sub(out=s_[:], in0=hs[:, sl], in1=c_[:])
                        nc.vector.tensor_mul(out=t_[:], in0=z[:], in1=s_[:])
                        nc.vector.tensor_add(out=hs[:, sl1], in0=c_[:], in1=t_[:])
                        if t < T - 1:
                            nc.sync.dma_start(out=xh_A[64:128, sl1], in_=hs[0:64, sl1])
                            nc.sync.dma_start(out=xh_B[64:128, sl1], in_=hs[64:128, sl1])
                    nc.sync.dma_start(out=hs_out[w, p], in_=hs[:, C:SLOTS])
    nc.compile()
    return nc


def _prep_inputs(x0, x1, x2, gru_wih, gru_whh, gru_bih, gru_bhh):
    xs = np.stack([x0, x1, x2])  # [W, N, T, D]
    xpad = np.zeros((W, NPAD, T, D), np.float32)
    xpad[:, :N] = xs
    # per-core x: [W, NPAIR, 2, 64, T*C]
    in_maps = []
    # weights: lhsT per gate: [K=128, M=64]
    wl = np.zeros((128, W * 4 * 64), np.float32)
    bl = np.zeros((128, W * 4), np.float32)
    for w in range(W):
        wih, whh = gru_wih[w], gru_whh[w]        # [3H, D], [3H, H]
        bih, bhh = gru_bih[w], gru_bhh[w]
        for g, (top, bot, bias) in enumerate([
                (wih[0:64], whh[0:64], bih[0:64] + bhh[0:64]),          # r
                (wih[64:128], whh[64:128], bih[64:128] + bhh[64:128]),  # z
                (np.zeros((64, 64), np.float32), whh[128:192], bhh[128:192]),  # hn
                (wih[128:192], np.zeros((64, 64), np.float32), bih[128:192]),  # xn
        ]):
            col = (w * 4 + g) * 64
            wl[0:64, col:col + 64] = top.T
            wl[64:128, col:col + 64] = bot.T
            bl[0:64, w * 4 + g] = bias
            bl[64:128, w * 4 + g] = bias
    for cid in range(NCORES):
        sl = xpad[:, cid * PER_CORE:(cid + 1) * PER_CORE]  # [W, 3072, T, D]
        xc = np.zeros((W, NPAIR, 2, 64, T * C), np.float32)
        for p in range(NPAIR):
            for hfl in range(2):
                blk = sl[:, (p * 2 + hfl) * C:(p * 2 + hfl + 1) * C]  # [W,C,T,D]
                xc[:, p, hfl] = blk.transpose(0, 3, 2, 1).reshape(W, 64, T * C)
        in_maps.append({"x": xc, "wl": wl, "bl": bl})
    return in_maps


def _np_attn(seq, w, b):
    st = np.swapaxes(seq, 1, 2)
    e = st @ w.T + b
    e = e - e.max(-1, keepdims=True)
    p = np.exp(e)
    p = p / p.sum(-1, keepdims=True)
    return np.sum(np.swapaxes(p, 1, 2) * seq, axis=1)


def kernel(x0, x1, x2, gru_wih, gru_whh, gru_bih, gru_bhh, att_w, att_b,
           ww_w, ww_b, gat_w, gat_att_src, gat_att_dst, gat_b,
           fus_w, fus_b, reg_w, reg_b, cls_w, cls_b, edge_index):
    if "nc" not in _cache:
        _cache["nc"] = _build_program()
    nc = _cache["nc"]
    in_maps = _prep_inputs(x0, x1, x2, gru_wih, gru_whh, gru_bih, gru_bhh)
    res = run_bass_kernel_spmd(nc, in_maps, list(range(NCORES)))
    _cache["exec_ns"] = res.exec_time_ns
    # reassemble hs: [W, N, T, H]
    hs = np.zeros((W, NPAD, T, H), np.float32)
    for cid in range(NCORES):
        h = res.results[cid]["hs"]  # [W, NPAIR, 128, T*C]
        h = h.reshape(W, NPAIR, 128, T, C)
        for p in range(NPAIR):
            base = cid * PER_CORE + p * 2 * C
            hs[:, base:base + C] = h[:, p, 0:64].transpose(0, 3, 2, 1)
            hs[:, base + C:base + 2 * C] = h[:, p, 64:128].transpose(0, 3, 2, 1)
    hs = hs[:, :N]  # [W, N, T, H]

    # host: attention blocks + GAT + fusion (numpy)
    emb = np.stack([_np_attn(hs[w], att_w[w], att_b[w]) for w in range(W)])
    emb = np.swapaxes(emb, 0, 1)                  # (N, W, H)
    weekly = _np_attn(emb, ww_w, ww_b)            # (N, H)

    xg = weekly @ gat_w.T
    loops = np.arange(N, dtype=edge_index.dtype)
    src = np.concatenate([edge_index[0], loops])
    dst = np.concatenate([edge_index[1], loops])
    a = xg @ gat_att_src + 0.0
    ad = xg @ gat_att_dst
    alpha = a[src] + ad[dst]
    alpha = np.where(alpha > 0, alpha, 0.2 * alpha)
    amax = np.full(N, -np.inf, np.float32)
    np.maximum.at(amax, dst, alpha)
    ex = np.exp(alpha - amax[dst])
    den = np.bincount(dst, weights=ex, minlength=N)
    coef = (ex / den[dst]).astype(np.float32)
    cat = np.zeros((N, H), np.float32)
    wsrc = coef[:, None] * xg[src]
    for f in range(H):
        cat[:, f] = np.bincount(dst, weights=wsrc[:, f], minlength=N)
    cat = cat + gat_b

    fus = np.concatenate([weekly, cat], axis=-1) @ fus_w.T + fus_b
    fus = np.maximum(fus, 0.0)
    reg = np.ravel(fus @ reg_w.T + reg_b)
    cls = np.ravel(1.0 / (1.0 + np.exp(-(fus @ cls_w.T + cls_b))))
    return (reg.astype(np.float32), cls.astype(np.float32))



# revision 2
# speedup vs baseline: 4.7748x; 4.7748x over previous
import sys
sys.path.insert(0, "/opt/trn_rl_repo")
import numpy as np
import ml_dtypes
import concourse.bass as bass
import concourse.bacc as bacc
import concourse.mybir as mybir
import concourse.tile as tile
from concourse.bass_utils import run_bass_kernel_spmd

# Problem constants (hardcoded per contract)
N = 20000
T = 20
D = 64
H = 64
W = 3
NCORES = 8
NPAD = 24576            # 8 * 3072
PER_CORE = 3072         # padded per-core stocks
C = 512                 # chunk size
NPAIR = 3               # pairs of chunks per core (6 chunks)
dt = mybir.dt
BF16 = ml_dtypes.bfloat16

_cache = {}


def _build_program():
    """Bass program: per-week GRU + T-attention, then weekly attention.

    x arrives bf16 in natural node-major layout [W, PER_CORE, T*D] and is
    transposed on-device with XBAR DMA transposes of [512, 128] blocks: a
    block covers a (t=2k, t=2k+1) pair (contiguous 128 cols), transposing
    to [128, 512] with even-t features on partitions 0:64, odd-t on 64:128.

    GRU slot layout alternates by parity so no partition-shift is needed
    for x: even slot t has [x_t; h_{t-1}], odd slot t has [h_{t-1}; x_t];
    lhsT weights come in even/odd row-swapped variants.

    Attention (softmax over T without max-subtraction — |e| <= ~11 so exp
    is safe in f32) and the weekly W=3 attention run on-device; only the
    weekly embedding [NPAIR, 128, C] goes back to DRAM.
    """
    nc = bacc.Bacc("TRN2", target_bir_lowering=False, debug=False,
                   num_devices=NCORES)
    SLOTS = 21 * C
    x_in = nc.declare_dram_parameter("x", [W, PER_CORE, T * D], dt.bfloat16,
                                     isOutput=False)
    wl_in = nc.declare_dram_parameter("wl", [128, W * 4 * 2 * 64], dt.float32,
                                      isOutput=False)
    bl_in = nc.declare_dram_parameter("bl", [128, W * 4], dt.float32,
                                      isOutput=False)
    aw_in = nc.declare_dram_parameter("aw", [128, W * T * T], dt.float32,
                                      isOutput=False)
    ab_in = nc.declare_dram_parameter("ab", [128, W * T], dt.float32,
                                      isOutput=False)
    vw_in = nc.declare_dram_parameter("vw", [128, W * W], dt.float32,
                                      isOutput=False)
    vb_in = nc.declare_dram_parameter("vb", [128, W], dt.float32,
                                      isOutput=False)
    wk_out = nc.declare_dram_parameter("wk", [NPAIR, 128, C], dt.float32,
                                       isOutput=True)
    AF = mybir.ActivationFunctionType
    OP = mybir.AluOpType

    with tile.TileContext(nc) as tc:
        with tc.tile_pool(name="wpool", bufs=1) as wpool, \
             tc.tile_pool(name="xbfp", bufs=2) as xbfp, \
             tc.tile_pool(name="xh", bufs=1) as xhp, \
             tc.tile_pool(name="hsp", bufs=1) as hsp, \
             tc.tile_pool(name="gate", bufs=2) as gp, \
             tc.tile_pool(name="attn", bufs=1) as ap_, \
             tc.tile_pool(name="ep", bufs=2) as ep, \
             tc.tile_pool(name="psum", bufs=2, space="PSUM") as pp:
            wl = wpool.tile([128, W * 4 * 2 * 64], dt.float32)
            bl = wpool.tile([128, W * 4], dt.float32)
            aw = wpool.tile([128, W * T * T], dt.float32)
            ab = wpool.tile([128, W * T], dt.float32)
            vw = wpool.tile([128, W * W], dt.float32)
            vb = wpool.tile([128, W], dt.float32)
            nc.sync.dma_start(out=wl[:], in_=wl_in[:, :])
            nc.sync.dma_start(out=bl[:], in_=bl_in[:, :])
            nc.sync.dma_start(out=aw[:], in_=aw_in[:, :])
            nc.sync.dma_start(out=ab[:], in_=ab_in[:, :])
            nc.sync.dma_start(out=vw[:], in_=vw_in[:, :])
            nc.sync.dma_start(out=vb[:], in_=vb_in[:, :])

            for p in range(NPAIR):
                embs = []
                for w in range(W):
                    xh_A = xhp.tile([128, SLOTS], dt.float32, tag="xha")
                    xh_B = xhp.tile([128, SLOTS], dt.float32, tag="xhb")
                    hs = hsp.tile([128, SLOTS], dt.float32, tag="hs")
                    # --- load x via XBAR transpose-DMA, upcast to f32 ---
                    for half, tgt in ((0, xh_A), (1, xh_B)):
                        base = p * 2 * C + half * C
                        for tb in range(T // 2):
                            xbf = xbfp.tile([128, C], dt.bfloat16,
                                            tag=f"xbf{half}")
                            nc.sync.dma_start(
                                out=xbf[:],
                                in_=x_in[w, base:base + C,
                                         tb * 128:(tb + 1) * 128],
                                transpose=True)
                            te, to = 2 * tb, 2 * tb + 1
                            nc.scalar.copy(
                                out=tgt[0:64, te * C:(te + 1) * C],
                                in_=xbf[0:64, :])
                            nc.scalar.copy(
                                out=tgt[64:128, to * C:(to + 1) * C],
                                in_=xbf[64:128, :])
                    # h_{-1} = 0 (slot 0 is even: h on rows 64:128)
                    nc.vector.memset(xh_A[64:128, 0:C], 0.0)
                    nc.vector.memset(xh_B[64:128, 0:C], 0.0)
                    nc.vector.memset(hs[:, 0:C], 0.0)
                    # --- GRU recurrence ---
                    for t in range(T):
                        par = t % 2
                        sl = slice(t * C, (t + 1) * C)
                        sl1 = slice((t + 1) * C, (t + 2) * C)
                        ps = []
                        for g in range(4):
                            pg = pp.tile([128, C], dt.float32, tag=f"g{g}")
                            col = ((w * 4 + g) * 2 + par) * 64
                            lt = wl[:, col:col + 64]
                            nc.tensor.matmul(out=pg[0:64, :], lhsT=lt,
                                             rhs=xh_A[:, sl],
                                             start=True, stop=True)
                            nc.tensor.matmul(out=pg[64:128, :], lhsT=lt,
                                             rhs=xh_B[:, sl],
                                             start=True, stop=True,
                                             tile_position=(0, 64))
                            ps.append(pg)
                        r = gp.tile([128, C], dt.float32, tag="r")
                        z = gp.tile([128, C], dt.float32, tag="z")
                        v = gp.tile([128, C], dt.float32, tag="v")
                        wo = gp.tile([128, C], dt.float32, tag="wo")
                        c_ = gp.tile([128, C], dt.float32, tag="c")
                        s_ = gp.tile([128, C], dt.float32, tag="s")
                        t_ = gp.tile([128, C], dt.float32, tag="t")
                        nc.scalar.activation(out=r[:], in_=ps[0][:],
                                             func=AF.Sigmoid,
                                             bias=bl[:, (w * 4):(w * 4 + 1)])
                        nc.scalar.activation(out=z[:], in_=ps[1][:],
                                             func=AF.Sigmoid,
                                             bias=bl[:, (w * 4 + 1):(w * 4 + 2)])
                        # v = (hn + b_hn) * r ; wo = (xn + b_in) + v
                        nc.vector.scalar_tensor_tensor(
                            out=v[:], in0=ps[2][:],
                            scalar=bl[:, (w * 4 + 2):(w * 4 + 3)],
                            in1=r[:], op0=OP.add, op1=OP.mult)
                        nc.vector.scalar_tensor_tensor(
                            out=wo[:], in0=ps[3][:],
                            scalar=bl[:, (w * 4 + 3):(w * 4 + 4)],
                            in1=v[:], op0=OP.add, op1=OP.add)
                        nc.scalar.activation(out=c_[:], in_=wo[:], func=AF.Tanh)
                        nc.vector.tensor_sub(out=s_[:], in0=hs[:, sl], in1=c_[:])
                        nc.vector.tensor_mul(out=t_[:], in0=z[:], in1=s_[:])
                        nc.vector.tensor_add(out=hs[:, sl1], in0=c_[:], in1=t_[:])
                        if t < T - 1:
                            if (t + 1) % 2 == 1:
                                # odd slot: h on rows 0:64
                                nc.sync.dma_start(out=xh_A[0:64, sl1],
                                                  in_=hs[0:64, sl1])
                                nc.sync.dma_start(out=xh_B[0:64, sl1],
                                                  in_=hs[64:128, sl1])
                            else:
                                # even slot: h on rows 64:128
                                nc.sync.dma_start(out=xh_A[64:128, sl1],
                                                  in_=hs[0:64, sl1])
                                nc.sync.dma_start(out=xh_B[64:128, sl1],
                                                  in_=hs[64:128, sl1])
                    # --- attention over T (softmax over s, streamed) ---
                    den = ap_.tile([128, C], dt.float32, tag="den")
                    acc = ap_.tile([128, C], dt.float32, tag="acc")
                    tmp = ap_.tile([128, C], dt.float32, tag="tmp")
                    for s in range(T):
                        e = ep.tile([128, C], dt.float32, tag="e")
                        a0 = w * T * T + s * T
                        nc.vector.tensor_scalar_mul(
                            out=e[:], in0=hs[:, C:2 * C],
                            scalar1=aw[:, a0:a0 + 1])
                        for t in range(1, T):
                            nc.vector.scalar_tensor_tensor(
                                out=e[:], in0=hs[:, (t + 1) * C:(t + 2) * C],
                                scalar=aw[:, a0 + t:a0 + t + 1],
                                in1=e[:], op0=OP.mult, op1=OP.add)
                        nc.scalar.activation(out=e[:], in_=e[:], func=AF.Exp,
                                             bias=ab[:, w * T + s:w * T + s + 1])
                        if s == 0:
                            nc.vector.tensor_copy(out=den[:], in_=e[:])
                            nc.vector.tensor_mul(out=acc[:], in0=e[:],
                                                 in1=hs[:, C:2 * C])
                        else:
                            nc.vector.tensor_add(out=den[:], in0=den[:], in1=e[:])
                            nc.vector.tensor_mul(
                                out=tmp[:], in0=e[:],
                                in1=hs[:, (s + 1) * C:(s + 2) * C])
                            nc.vector.tensor_add(out=acc[:], in0=acc[:],
                                                 in1=tmp[:])
                    rcp = ap_.tile([128, C], dt.float32, tag="rcp")
                    nc.vector.reciprocal(out=rcp[:], in_=den[:])
                    emb = ap_.tile([128, C], dt.float32, tag=f"emb{w}")
                    nc.vector.tensor_mul(out=emb[:], in0=acc[:], in1=rcp[:])
                    embs.append(emb)
                # --- weekly attention over W=3 ---
                wden = ap_.tile([128, C], dt.float32, tag="wden")
                wacc = ap_.tile([128, C], dt.float32, tag="wacc")
                wtmp = ap_.tile([128, C], dt.float32, tag="wtmp")
                for vv in range(W):
                    we = ep.tile([128, C], dt.float32, tag="we")
                    nc.vector.tensor_scalar_mul(
                        out=we[:], in0=embs[0][:],
                        scalar1=vw[:, vv * W:vv * W + 1])
                    for u in range(1, W):
                        nc.vector.scalar_tensor_tensor(
                            out=we[:], in0=embs[u][:],
                            scalar=vw[:, vv * W + u:vv * W + u + 1],
                            in1=we[:], op0=OP.mult, op1=OP.add)
                    nc.scalar.activation(out=we[:], in_=we[:], func=AF.Exp,
                                         bias=vb[:, vv:vv + 1])
                    if vv == 0:
                        nc.vector.tensor_copy(out=wden[:], in_=we[:])
                        nc.vector.tensor_mul(out=wacc[:], in0=we[:],
                                             in1=embs[0][:])
                    else:
                        nc.vector.tensor_add(out=wden[:], in0=wden[:], in1=we[:])
                        nc.vector.tensor_mul(out=wtmp[:], in0=we[:],
                                             in1=embs[vv][:])
                        nc.vector.tensor_add(out=wacc[:], in0=wacc[:],
                                             in1=wtmp[:])
                wrcp = ap_.tile([128, C], dt.float32, tag="wrcp")
                nc.vector.reciprocal(out=wrcp[:], in_=wden[:])
                wk = ap_.tile([128, C], dt.float32, tag="wk")
                nc.vector.tensor_mul(out=wk[:], in0=wacc[:], in1=wrcp[:])
                nc.sync.dma_start(out=wk_out[p], in_=wk[:])
    nc.compile()
    return nc


def _prep_weights(gru_wih, gru_whh, gru_bih, gru_bhh, att_w, att_b, ww_w, ww_b):
    # lhsT per (week, gate, parity): [K=128, M=64]; odd parity swaps halves
    wl = np.zeros((128, W * 4 * 2 * 64), np.float32)
    bl = np.zeros((128, W * 4), np.float32)
    for w in range(W):
        wih, whh = gru_wih[w], gru_whh[w]        # [3H, D], [3H, H]
        bih, bhh = gru_bih[w], gru_bhh[w]
        zero = np.zeros((64, 64), np.float32)
        for g, (xw, hw, bias) in enumerate([
                (wih[0:64], whh[0:64], bih[0:64] + bhh[0:64]),          # r
                (wih[64:128], whh[64:128], bih[64:128] + bhh[64:128]),  # z
                (zero, whh[128:192], bhh[128:192]),                     # hn
                (wih[128:192], zero, bih[128:192]),                     # xn
        ]):
            ce = ((w * 4 + g) * 2) * 64
            co = ce + 64
            wl[0:64, ce:ce + 64] = xw.T
            wl[64:128, ce:ce + 64] = hw.T
            wl[0:64, co:co + 64] = hw.T
            wl[64:128, co:co + 64] = xw.T
            bl[0:64, w * 4 + g] = bias
            bl[64:128, w * 4 + g] = bias
    aw = np.ascontiguousarray(np.broadcast_to(
        att_w.reshape(1, W * T * T), (128, W * T * T)), dtype=np.float32)
    ab = np.ascontiguousarray(np.broadcast_to(
        att_b.reshape(1, W * T), (128, W * T)), dtype=np.float32)
    vw = np.ascontiguousarray(np.broadcast_to(
        ww_w.reshape(1, W * W), (128, W * W)), dtype=np.float32)
    vb = np.ascontiguousarray(np.broadcast_to(
        ww_b.reshape(1, W), (128, W)), dtype=np.float32)
    return wl, bl, aw, ab, vw, vb


def kernel(x0, x1, x2, gru_wih, gru_whh, gru_bih, gru_bhh, att_w, att_b,
           ww_w, ww_b, gat_w, gat_att_src, gat_att_dst, gat_b,
           fus_w, fus_b, reg_w, reg_b, cls_w, cls_b, edge_index):
    if "nc" not in _cache:
        _cache["nc"] = _build_program()
    nc = _cache["nc"]
    wl, bl, aw, ab, vw, vb = _prep_weights(
        gru_wih, gru_whh, gru_bih, gru_bhh, att_w, att_b, ww_w, ww_b)
    # x in natural layout, bf16, interleaved [core, week]
    xg = np.zeros((NCORES * W, PER_CORE, T * D), BF16)
    for c in range(NCORES):
        lo = c * PER_CORE
        hi = min(N, lo + PER_CORE)
        if lo >= N:
            continue
        for w, xw in enumerate((x0, x1, x2)):
            xg[c * W + w, 0:hi - lo] = xw[lo:hi].reshape(hi - lo, T * D)
    in_maps = [{"x": xg[c * W:(c + 1) * W], "wl": wl, "bl": bl,
                "aw": aw, "ab": ab, "vw": vw, "vb": vb}
               for c in range(NCORES)]
    res = run_bass_kernel_spmd(nc, in_maps, list(range(NCORES)))
    _cache["exec_ns"] = res.exec_time_ns
    # weekly: [8, NPAIR, 128, C] -> (N, H)
    wk = np.stack([res.results[c]["wk"] for c in range(NCORES)])
    weekly = wk.reshape(NCORES, NPAIR, 2, H, C).transpose(0, 1, 2, 4, 3) \
               .reshape(NPAD, H)[:N]

    # --- GAT on host (sorted-edge segment ops) ---
    xgat = weekly @ gat_w.T
    asrc = xgat @ gat_att_src
    adst = xgat @ gat_att_dst
    loops = np.arange(N, dtype=edge_index.dtype)
    src = np.concatenate([edge_index[0], loops])
    dst = np.concatenate([edge_index[1], loops])
    perm = np.argsort(dst)
    ds, ss = dst[perm], src[perm]
    counts = np.bincount(ds, minlength=N)
    starts = np.zeros(N, np.int64)
    np.cumsum(counts[:-1], out=starts[1:])
    alpha = asrc[ss] + adst[ds]
    alpha = np.where(alpha > 0, alpha, 0.2 * alpha)
    amax = np.maximum.reduceat(alpha, starts)
    ex = np.exp(alpha - np.repeat(amax, counts))
    den = np.add.reduceat(ex, starts)
    coef = (ex / np.repeat(den, counts)).astype(np.float32)
    cat = np.add.reduceat(coef[:, None] * xgat[ss], starts, axis=0) + gat_b

    fus = np.concatenate([weekly, cat], axis=-1) @ fus_w.T + fus_b
    fus = np.maximum(fus, 0.0)
    reg = np.ravel(fus @ reg_w.T + reg_b)
    cls = np.ravel(1.0 / (1.0 + np.exp(-(fus @ cls_w.T + cls_b))))
    return (reg.astype(np.float32), cls.astype(np.float32))


# revision 5
# speedup vs baseline: 5.4879x; 1.1494x over previous
import sys
sys.path.insert(0, "/opt/trn_rl_repo")
import numpy as np
import ml_dtypes
import concourse.bass as bass
import concourse.bacc as bacc
import concourse.mybir as mybir
import concourse.tile as tile
from concourse.bass_utils import run_bass_kernel_spmd
from scipy.sparse import csr_matrix

# Problem constants (hardcoded per contract)
N = 20000
T = 20
D = 64
H = 64
W = 3
NCORES = 8
NPAD = 20480            # 8 * 2560
PER_CORE = 2560         # padded per-core stocks (5 chunks of 512)
C = 512                 # chunk size
NPAIR = 3               # chunk pairs per core: (0,1), (2,3), (4, zero)
PAIRS = ((0, 1), (2, 3), (4, None))
dt = mybir.dt
BF16 = ml_dtypes.bfloat16

_cache = {}


def _build_program():
    """Bass program: per-week GRU + T-attention, then weekly attention.

    x arrives bf16 in natural node-major layout [W, PER_CORE, T*D] and is
    transposed on-device with XBAR DMA transposes of [512, 128] blocks: a
    block covers a (t=2k, t=2k+1) pair (contiguous 128 cols), transposing
    to [128, 512] with even-t features on partitions 0:64, odd-t on 64:128.

    GRU slot layout alternates by parity so no partition-shift is needed
    for x: even slot t has [x_t; h_{t-1}], odd slot t has [h_{t-1}; x_t];
    lhsT weights come in even/odd row-swapped variants.

    Attention (softmax over T without max-subtraction — |e| <= ~11 so exp
    is safe in f32) and the weekly W=3 attention run on-device; only the
    weekly embedding [NPAIR, 128, C] goes back to DRAM.
    """
    nc = bacc.Bacc("TRN2", target_bir_lowering=False, debug=False,
                   num_devices=NCORES)
    SLOTS = 21 * C
    x_in = nc.declare_dram_parameter("x", [W, PER_CORE, T * D], dt.bfloat16,
                                     isOutput=False)
    wl_in = nc.declare_dram_parameter("wl", [128, W * 4 * 2 * 64], dt.float32,
                                      isOutput=False)
    bl_in = nc.declare_dram_parameter("bl", [128, W * 4], dt.float32,
                                      isOutput=False)
    aw_in = nc.declare_dram_parameter("aw", [128, W * T * T], dt.float32,
                                      isOutput=False)
    ab_in = nc.declare_dram_parameter("ab", [128, W * T], dt.float32,
                                      isOutput=False)
    vw_in = nc.declare_dram_parameter("vw", [128, W * W], dt.float32,
                                      isOutput=False)
    vb_in = nc.declare_dram_parameter("vb", [128, W], dt.float32,
                                      isOutput=False)
    wk_out = nc.declare_dram_parameter("wk", [NPAIR, 128, C], dt.float32,
                                       isOutput=True)
    AF = mybir.ActivationFunctionType
    OP = mybir.AluOpType

    with tile.TileContext(nc) as tc:
        with tc.tile_pool(name="wpool", bufs=1) as wpool, \
             tc.tile_pool(name="xbfp", bufs=2) as xbfp, \
             tc.tile_pool(name="xh", bufs=1) as xhp, \
             tc.tile_pool(name="hsp", bufs=1) as hsp, \
             tc.tile_pool(name="gate", bufs=2) as gp, \
             tc.tile_pool(name="attn", bufs=1) as ap_, \
             tc.tile_pool(name="ep", bufs=2) as ep, \
             tc.tile_pool(name="psum", bufs=2, space="PSUM") as pp:
            wl = wpool.tile([128, W * 4 * 2 * 64], dt.float32)
            bl = wpool.tile([128, W * 4], dt.float32)
            aw = wpool.tile([128, W * T * T], dt.float32)
            ab = wpool.tile([128, W * T], dt.float32)
            vw = wpool.tile([128, W * W], dt.float32)
            vb = wpool.tile([128, W], dt.float32)
            nc.sync.dma_start(out=wl[:], in_=wl_in[:, :])
            nc.sync.dma_start(out=bl[:], in_=bl_in[:, :])
            nc.sync.dma_start(out=aw[:], in_=aw_in[:, :])
            nc.sync.dma_start(out=ab[:], in_=ab_in[:, :])
            nc.sync.dma_start(out=vw[:], in_=vw_in[:, :])
            nc.sync.dma_start(out=vb[:], in_=vb_in[:, :])

            for p, (ca, cb) in enumerate(PAIRS):
                embs = []
                for w in range(W):
                    xh_A = xhp.tile([128, SLOTS], dt.float32, tag="xha")
                    xh_B = xhp.tile([128, SLOTS], dt.float32, tag="xhb")
                    hs = hsp.tile([128, SLOTS], dt.float32, tag="hs")
                    # --- load x via XBAR transpose-DMA, upcast to f32 ---
                    for half, (ck, tgt) in enumerate(((ca, xh_A), (cb, xh_B))):
                        if ck is None:
                            nc.vector.memset(tgt[:, :], 0.0)
                            continue
                        base = ck * C
                        for tb in range(T // 2):
                            xbf = xbfp.tile([128, C], dt.bfloat16,
                                            tag=f"xbf{half}")
                            nc.sync.dma_start(
                                out=xbf[:],
                                in_=x_in[w, base:base + C,
                                         tb * 128:(tb + 1) * 128],
                                transpose=True)
                            te, to = 2 * tb, 2 * tb + 1
                            nc.scalar.copy(
                                out=tgt[0:64, te * C:(te + 1) * C],
                                in_=xbf[0:64, :])
                            nc.scalar.copy(
                                out=tgt[64:128, to * C:(to + 1) * C],
                                in_=xbf[64:128, :])
                    # h_{-1} = 0 (slot 0 is even: h on rows 64:128)
                    nc.vector.memset(xh_A[64:128, 0:C], 0.0)
                    if cb is not None:
                        nc.vector.memset(xh_B[64:128, 0:C], 0.0)
                    nc.vector.memset(hs[:, 0:C], 0.0)
                    # --- GRU recurrence ---
                    for t in range(T):
                        par = t % 2
                        sl = slice(t * C, (t + 1) * C)
                        sl1 = slice((t + 1) * C, (t + 2) * C)
                        ps = []
                        for g in range(4):
                            pg = pp.tile([128, C], dt.float32, tag=f"g{g}")
                            col = ((w * 4 + g) * 2 + par) * 64
                            lt = wl[:, col:col + 64]
                            nc.tensor.matmul(out=pg[0:64, :], lhsT=lt,
                                             rhs=xh_A[:, sl],
                                             start=True, stop=True)
                            nc.tensor.matmul(out=pg[64:128, :], lhsT=lt,
                                             rhs=xh_B[:, sl],
                                             start=True, stop=True,
                                             tile_position=(0, 64))
                            ps.append(pg)
                        r = gp.tile([128, C], dt.float32, tag="r")
                        z = gp.tile([128, C], dt.float32, tag="z")
                        v = gp.tile([128, C], dt.float32, tag="v")
                        wo = gp.tile([128, C], dt.float32, tag="wo")
                        c_ = gp.tile([128, C], dt.float32, tag="c")
                        s_ = gp.tile([128, C], dt.float32, tag="s")
                        t_ = gp.tile([128, C], dt.float32, tag="t")
                        nc.scalar.activation(out=r[:], in_=ps[0][:],
                                             func=AF.Sigmoid,
                                             bias=bl[:, (w * 4):(w * 4 + 1)])
                        nc.scalar.activation(out=z[:], in_=ps[1][:],
                                             func=AF.Sigmoid,
                                             bias=bl[:, (w * 4 + 1):(w * 4 + 2)])
                        # v = (hn + b_hn) * r ; wo = (xn + b_in) + v
                        nc.vector.scalar_tensor_tensor(
                            out=v[:], in0=ps[2][:],
                            scalar=bl[:, (w * 4 + 2):(w * 4 + 3)],
                            in1=r[:], op0=OP.add, op1=OP.mult)
                        nc.vector.scalar_tensor_tensor(
                            out=wo[:], in0=ps[3][:],
                            scalar=bl[:, (w * 4 + 3):(w * 4 + 4)],
                            in1=v[:], op0=OP.add, op1=OP.add)
                        nc.scalar.activation(out=c_[:], in_=wo[:], func=AF.Tanh)
                        nc.vector.tensor_sub(out=s_[:], in0=hs[:, sl], in1=c_[:])
                        nc.vector.tensor_mul(out=t_[:], in0=z[:], in1=s_[:])
                        nc.vector.tensor_add(out=hs[:, sl1], in0=c_[:], in1=t_[:])
                        if t < T - 1:
                            if (t + 1) % 2 == 1:
                                # odd slot: h on rows 0:64
                                nc.sync.dma_start(out=xh_A[0:64, sl1],
                                                  in_=hs[0:64, sl1])
                                nc.sync.dma_start(out=xh_B[0:64, sl1],
                                                  in_=hs[64:128, sl1])
                            else:
                                # even slot: h on rows 64:128
                                nc.sync.dma_start(out=xh_A[64:128, sl1],
                                                  in_=hs[0:64, sl1])
                                nc.sync.dma_start(out=xh_B[64:128, sl1],
                                                  in_=hs[64:128, sl1])
                    # --- attention over T (softmax over s, streamed) ---
                    den = ap_.tile([128, C], dt.float32, tag="den")
                    acc = ap_.tile([128, C], dt.float32, tag="acc")
                    tmp = ap_.tile([128, C], dt.float32, tag="tmp")
                    for s in range(T):
                        e = ep.tile([128, C], dt.float32, tag="e")
                        a0 = w * T * T + s * T
                        nc.vector.tensor_scalar_mul(
                            out=e[:], in0=hs[:, C:2 * C],
                            scalar1=aw[:, a0:a0 + 1])
                        for t in range(1, T):
                            nc.vector.scalar_tensor_tensor(
                                out=e[:], in0=hs[:, (t + 1) * C:(t + 2) * C],
                                scalar=aw[:, a0 + t:a0 + t + 1],
                                in1=e[:], op0=OP.mult, op1=OP.add)
                        nc.scalar.activation(out=e[:], in_=e[:], func=AF.Exp,
                                             bias=ab[:, w * T + s:w * T + s + 1])
                        if s == 0:
                            nc.vector.tensor_copy(out=den[:], in_=e[:])
                            nc.vector.tensor_mul(out=acc[:], in0=e[:],
                                                 in1=hs[:, C:2 * C])
                        else:
                            nc.vector.tensor_add(out=den[:], in0=den[:], in1=e[:])
                            nc.vector.tensor_mul(
                                out=tmp[:], in0=e[:],
                                in1=hs[:, (s + 1) * C:(s + 2) * C])
                            nc.vector.tensor_add(out=acc[:], in0=acc[:],
                                                 in1=tmp[:])
                    rcp = ap_.tile([128, C], dt.float32, tag="rcp")
                    nc.vector.reciprocal(out=rcp[:], in_=den[:])
                    emb = ap_.tile([128, C], dt.float32, tag=f"emb{w}")
                    nc.vector.tensor_mul(out=emb[:], in0=acc[:], in1=rcp[:])
                    embs.append(emb)
                # --- weekly attention over W=3 ---
                wden = ap_.tile([128, C], dt.float32, tag="wden")
                wacc = ap_.tile([128, C], dt.float32, tag="wacc")
                wtmp = ap_.tile([128, C], dt.float32, tag="wtmp")
                for vv in range(W):
                    we = ep.tile([128, C], dt.float32, tag="we")
                    nc.vector.tensor_scalar_mul(
                        out=we[:], in0=embs[0][:],
                        scalar1=vw[:, vv * W:vv * W + 1])
                    for u in range(1, W):
                        nc.vector.scalar_tensor_tensor(
                            out=we[:], in0=embs[u][:],
                            scalar=vw[:, vv * W + u:vv * W + u + 1],
                            in1=we[:], op0=OP.mult, op1=OP.add)
                    nc.scalar.activation(out=we[:], in_=we[:], func=AF.Exp,
                                         bias=vb[:, vv:vv + 1])
                    if vv == 0:
                        nc.vector.tensor_copy(out=wden[:], in_=we[:])
                        nc.vector.tensor_mul(out=wacc[:], in0=we[:],
                                             in1=embs[0][:])
                    else:
                        nc.vector.tensor_add(out=wden[:], in0=wden[:], in1=we[:])
                        nc.vector.tensor_mul(out=wtmp[:], in0=we[:],
                                             in1=embs[vv][:])
                        nc.vector.tensor_add(out=wacc[:], in0=wacc[:],
                                             in1=wtmp[:])
                wrcp = ap_.tile([128, C], dt.float32, tag="wrcp")
                nc.vector.reciprocal(out=wrcp[:], in_=wden[:])
                wk = ap_.tile([128, C], dt.float32, tag="wk")
                nc.vector.tensor_mul(out=wk[:], in0=wacc[:], in1=wrcp[:])
                nc.sync.dma_start(out=wk_out[p], in_=wk[:])
    nc.compile()
    return nc


def _prep_weights(gru_wih, gru_whh, gru_bih, gru_bhh, att_w, att_b, ww_w, ww_b):
    # lhsT per (week, gate, parity): [K=128, M=64]; odd parity swaps halves
    wl = np.zeros((128, W * 4 * 2 * 64), np.float32)
    bl = np.zeros((128, W * 4), np.float32)
    for w in range(W):
        wih, whh = gru_wih[w], gru_whh[w]        # [3H, D], [3H, H]
        bih, bhh = gru_bih[w], gru_bhh[w]
        zero = np.zeros((64, 64), np.float32)
        for g, (xw, hw, bias) in enumerate([
                (wih[0:64], whh[0:64], bih[0:64] + bhh[0:64]),          # r
                (wih[64:128], whh[64:128], bih[64:128] + bhh[64:128]),  # z
                (zero, whh[128:192], bhh[128:192]),                     # hn
                (wih[128:192], zero, bih[128:192]),                     # xn
        ]):
            ce = ((w * 4 + g) * 2) * 64
            co = ce + 64
            wl[0:64, ce:ce + 64] = xw.T
            wl[64:128, ce:ce + 64] = hw.T
            wl[0:64, co:co + 64] = hw.T
            wl[64:128, co:co + 64] = xw.T
            bl[0:64, w * 4 + g] = bias
            bl[64:128, w * 4 + g] = bias
    aw = np.ascontiguousarray(np.broadcast_to(
        att_w.reshape(1, W * T * T), (128, W * T * T)), dtype=np.float32)
    ab = np.ascontiguousarray(np.broadcast_to(
        att_b.reshape(1, W * T), (128, W * T)), dtype=np.float32)
    vw = np.ascontiguousarray(np.broadcast_to(
        ww_w.reshape(1, W * W), (128, W * W)), dtype=np.float32)
    vb = np.ascontiguousarray(np.broadcast_to(
        ww_b.reshape(1, W), (128, W)), dtype=np.float32)
    return wl, bl, aw, ab, vw, vb


def kernel(x0, x1, x2, gru_wih, gru_whh, gru_bih, gru_bhh, att_w, att_b,
           ww_w, ww_b, gat_w, gat_att_src, gat_att_dst, gat_b,
           fus_w, fus_b, reg_w, reg_b, cls_w, cls_b, edge_index):
    if "nc" not in _cache:
        _cache["nc"] = _build_program()
    nc = _cache["nc"]
    wl, bl, aw, ab, vw, vb = _prep_weights(
        gru_wih, gru_whh, gru_bih, gru_bhh, att_w, att_b, ww_w, ww_b)
    # x in natural layout, bf16, interleaved [core, week]
    xg = np.zeros((NCORES * W, PER_CORE, T * D), BF16)
    for c in range(NCORES):
        lo = c * PER_CORE
        hi = min(N, lo + PER_CORE)
        if lo >= N:
            continue
        for w, xw in enumerate((x0, x1, x2)):
            xg[c * W + w, 0:hi - lo] = xw[lo:hi].reshape(hi - lo, T * D)
    in_maps = [{"x": xg[c * W:(c + 1) * W], "wl": wl, "bl": bl,
                "aw": aw, "ab": ab, "vw": vw, "vb": vb}
               for c in range(NCORES)]
    res = run_bass_kernel_spmd(nc, in_maps, list(range(NCORES)))
    _cache["exec_ns"] = res.exec_time_ns
    # weekly: [8, NPAIR, 128, C]; per core the 6 half-chunks map to node
    # blocks 0..4 (the 6th is the zero pad of the solo pair)
    wk = np.stack([res.results[c]["wk"] for c in range(NCORES)])
    weekly = wk.reshape(NCORES, NPAIR, 2, H, C).transpose(0, 1, 2, 4, 3) \
               .reshape(NCORES, 6, C, H)[:, :5].reshape(NPAD, H)[:N]

    # --- GAT on host (sorted-edge segment softmax + CSR aggregate) ---
    xgat = weekly @ gat_w.T
    asrc = xgat @ gat_att_src
    adst = xgat @ gat_att_dst
    loops = np.arange(N, dtype=edge_index.dtype)
    src = np.concatenate([edge_index[0], loops])
    dst = np.concatenate([edge_index[1], loops])
    perm = np.argsort(dst)
    ds, ss = dst[perm], src[perm]
    counts = np.bincount(ds, minlength=N)
    indptr = np.zeros(N + 1, np.int64)
    np.cumsum(counts, out=indptr[1:])
    starts = indptr[:-1]
    alpha = asrc[ss] + adst[ds]
    alpha = np.where(alpha > 0, alpha, 0.2 * alpha)
    amax = np.maximum.reduceat(alpha, starts)
    ex = np.exp(alpha - np.repeat(amax, counts))
    den = np.add.reduceat(ex, starts)
    coef = (ex / np.repeat(den, counts)).astype(np.float32)
    cat = csr_matrix((coef, ss, indptr), shape=(N, N)) @ xgat + gat_b

    fus = np.concatenate([weekly, cat], axis=-1) @ fus_w.T + fus_b
    fus = np.maximum(fus, 0.0)
    reg = np.ravel(fus @ reg_w.T + reg_b)
    cls = np.ravel(1.0 / (1.0 + np.exp(-(fus @ cls_w.T + cls_b))))
    return (reg.astype(np.float32), cls.astype(np.float32))


# revision 6
# speedup vs baseline: 7.0602x; 1.2865x over previous
import sys
sys.path.insert(0, "/opt/trn_rl_repo")
import numpy as np
import ml_dtypes
import concourse.bass as bass
import concourse.bacc as bacc
import concourse.mybir as mybir
import concourse.tile as tile
from concourse.bass_utils import run_bass_kernel_spmd
from scipy.sparse import csr_matrix

# Problem constants (hardcoded per contract)
N = 20000
T = 20
D = 64
H = 64
W = 3
NCORES = 8
NPAD = 20480            # 8 * 2560
PER_CORE = 2560         # padded per-core stocks (5 chunks of 512)
C = 512                 # chunk size
NPAIR = 3               # chunk pairs per core: (0,1), (2,3), (4, zero)
PAIRS = ((0, 1), (2, 3), (4, None))
dt = mybir.dt
BF16 = ml_dtypes.bfloat16

_cache = {}


def _build_program():
    """Bass program: per-week GRU + T-attention, then weekly attention.

    x arrives bf16 in natural node-major layout [W, PER_CORE, T*D] and is
    transposed on-device with XBAR DMA transposes of [512, 128] blocks: a
    block covers a (t=2k, t=2k+1) pair (contiguous 128 cols), transposing
    to [128, 512] with even-t features on partitions 0:64, odd-t on 64:128.

    GRU slot layout alternates by parity so no partition-shift is needed
    for x: even slot t has [x_t; h_{t-1}], odd slot t has [h_{t-1}; x_t];
    lhsT weights come in even/odd row-swapped variants.

    Attention (softmax over T without max-subtraction — |e| <= ~11 so exp
    is safe in f32) and the weekly W=3 attention run on-device; only the
    weekly embedding [NPAIR, 128, C] goes back to DRAM.
    """
    nc = bacc.Bacc("TRN2", target_bir_lowering=False, debug=False,
                   num_devices=NCORES)
    SLOTS = 21 * C
    x_in = nc.declare_dram_parameter("x", [W, PER_CORE, T * D], dt.bfloat16,
                                     isOutput=False)
    wl_in = nc.declare_dram_parameter("wl", [128, W * 4 * 2 * 64], dt.float32,
                                      isOutput=False)
    bl_in = nc.declare_dram_parameter("bl", [128, W * 4], dt.float32,
                                      isOutput=False)
    aw_in = nc.declare_dram_parameter("aw", [128, W * T * T], dt.float32,
                                      isOutput=False)
    ab_in = nc.declare_dram_parameter("ab", [128, W * T], dt.float32,
                                      isOutput=False)
    vw_in = nc.declare_dram_parameter("vw", [128, W * W], dt.float32,
                                      isOutput=False)
    vb_in = nc.declare_dram_parameter("vb", [128, W], dt.float32,
                                      isOutput=False)
    wk_out = nc.declare_dram_parameter("wk", [NPAIR, 128, C], dt.float32,
                                       isOutput=True)
    AF = mybir.ActivationFunctionType
    OP = mybir.AluOpType

    with tile.TileContext(nc) as tc:
        with tc.tile_pool(name="wpool", bufs=1) as wpool, \
             tc.tile_pool(name="xbfp", bufs=2) as xbfp, \
             tc.tile_pool(name="xh", bufs=1) as xhp, \
             tc.tile_pool(name="hsp", bufs=1) as hsp, \
             tc.tile_pool(name="gate", bufs=2) as gp, \
             tc.tile_pool(name="attn", bufs=1) as ap_, \
             tc.tile_pool(name="ep", bufs=2) as ep, \
             tc.tile_pool(name="psum", bufs=2, space="PSUM") as pp:
            wl = wpool.tile([128, W * 4 * 2 * 64], dt.float32)
            bl = wpool.tile([128, W * 4], dt.float32)
            aw = wpool.tile([128, W * T * T], dt.float32)
            ab = wpool.tile([128, W * T], dt.float32)
            vw = wpool.tile([128, W * W], dt.float32)
            vb = wpool.tile([128, W], dt.float32)
            nc.sync.dma_start(out=wl[:], in_=wl_in[:, :])
            nc.sync.dma_start(out=bl[:], in_=bl_in[:, :])
            nc.sync.dma_start(out=aw[:], in_=aw_in[:, :])
            nc.sync.dma_start(out=ab[:], in_=ab_in[:, :])
            nc.sync.dma_start(out=vw[:], in_=vw_in[:, :])
            nc.sync.dma_start(out=vb[:], in_=vb_in[:, :])

            for p, (ca, cb) in enumerate(PAIRS):
                embs = []
                for w in range(W):
                    xh_A = xhp.tile([128, SLOTS], dt.float32, tag="xha")
                    xh_B = xhp.tile([128, SLOTS], dt.float32, tag="xhb")
                    hs = hsp.tile([128, SLOTS], dt.float32, tag="hs")
                    # --- load x via XBAR transpose-DMA, upcast to f32 ---
                    for half, (ck, tgt) in enumerate(((ca, xh_A), (cb, xh_B))):
                        if ck is None:
                            for i in range(T + 1):
                                nc.vector.memset(tgt[:, i * C:(i + 1) * C], 0.0)
                            continue
                        base = ck * C
                        for tb in range(T // 2):
                            xbf = xbfp.tile([128, C], dt.bfloat16,
                                            tag=f"xbf{half}")
                            nc.sync.dma_start(
                                out=xbf[:],
                                in_=x_in[w, base:base + C,
                                         tb * 128:(tb + 1) * 128],
                                transpose=True)
                            te, to = 2 * tb, 2 * tb + 1
                            nc.scalar.copy(
                                out=tgt[0:64, te * C:(te + 1) * C],
                                in_=xbf[0:64, :])
                            nc.scalar.copy(
                                out=tgt[64:128, to * C:(to + 1) * C],
                                in_=xbf[64:128, :])
                    # h_{-1} = 0 (slot 0 is even: h on rows 64:128)
                    nc.vector.memset(xh_A[64:128, 0:C], 0.0)
                    if cb is not None:
                        nc.vector.memset(xh_B[64:128, 0:C], 0.0)
                    nc.vector.memset(hs[:, 0:C], 0.0)
                    # --- GRU recurrence ---
                    for t in range(T):
                        par = t % 2
                        sl = slice(t * C, (t + 1) * C)
                        sl1 = slice((t + 1) * C, (t + 2) * C)
                        ps = []
                        for g in range(4):
                            pg = pp.tile([128, C], dt.float32, tag=f"g{g}")
                            col = ((w * 4 + g) * 2 + par) * 64
                            lt = wl[:, col:col + 64]
                            nc.tensor.matmul(out=pg[0:64, :], lhsT=lt,
                                             rhs=xh_A[:, sl],
                                             start=True, stop=True)
                            nc.tensor.matmul(out=pg[64:128, :], lhsT=lt,
                                             rhs=xh_B[:, sl],
                                             start=True, stop=True,
                                             tile_position=(0, 64))
                            ps.append(pg)
                        r = gp.tile([128, C], dt.float32, tag="r")
                        z = gp.tile([128, C], dt.float32, tag="z")
                        v = gp.tile([128, C], dt.float32, tag="v")
                        wo = gp.tile([128, C], dt.float32, tag="wo")
                        c_ = gp.tile([128, C], dt.float32, tag="c")
                        s_ = gp.tile([128, C], dt.float32, tag="s")
                        t_ = gp.tile([128, C], dt.float32, tag="t")
                        nc.scalar.activation(out=r[:], in_=ps[0][:],
                                             func=AF.Sigmoid,
                                             bias=bl[:, (w * 4):(w * 4 + 1)])
                        nc.scalar.activation(out=z[:], in_=ps[1][:],
                                             func=AF.Sigmoid,
                                             bias=bl[:, (w * 4 + 1):(w * 4 + 2)])
                        # v = (hn + b_hn) * r ; wo = (xn + b_in) + v
                        nc.vector.scalar_tensor_tensor(
                            out=v[:], in0=ps[2][:],
                            scalar=bl[:, (w * 4 + 2):(w * 4 + 3)],
                            in1=r[:], op0=OP.add, op1=OP.mult)
                        nc.vector.scalar_tensor_tensor(
                            out=wo[:], in0=ps[3][:],
                            scalar=bl[:, (w * 4 + 3):(w * 4 + 4)],
                            in1=v[:], op0=OP.add, op1=OP.add)
                        nc.scalar.activation(out=c_[:], in_=wo[:], func=AF.Tanh)
                        nc.vector.tensor_sub(out=s_[:], in0=hs[:, sl], in1=c_[:])
                        nc.vector.tensor_mul(out=t_[:], in0=z[:], in1=s_[:])
                        nc.vector.tensor_add(out=hs[:, sl1], in0=c_[:], in1=t_[:])
                        if t < T - 1:
                            if (t + 1) % 2 == 1:
                                # odd slot: h on rows 0:64
                                nc.sync.dma_start(out=xh_A[0:64, sl1],
                                                  in_=hs[0:64, sl1])
                                nc.sync.dma_start(out=xh_B[0:64, sl1],
                                                  in_=hs[64:128, sl1])
                            else:
                                # even slot: h on rows 64:128
                                nc.sync.dma_start(out=xh_A[64:128, sl1],
                                                  in_=hs[0:64, sl1])
                                nc.sync.dma_start(out=xh_B[64:128, sl1],
                                                  in_=hs[64:128, sl1])
                    # --- attention over T (softmax over s, streamed) ---
                    den = ap_.tile([128, C], dt.float32, tag="den")
                    acc = ap_.tile([128, C], dt.float32, tag="acc")
                    tmp = ap_.tile([128, C], dt.float32, tag="tmp")
                    for s in range(T):
                        e = ep.tile([128, C], dt.float32, tag="e")
                        a0 = w * T * T + s * T
                        nc.vector.tensor_scalar_mul(
                            out=e[:], in0=hs[:, C:2 * C],
                            scalar1=aw[:, a0:a0 + 1])
                        for t in range(1, T):
                            nc.vector.scalar_tensor_tensor(
                                out=e[:], in0=hs[:, (t + 1) * C:(t + 2) * C],
                                scalar=aw[:, a0 + t:a0 + t + 1],
                                in1=e[:], op0=OP.mult, op1=OP.add)
                        nc.scalar.activation(out=e[:], in_=e[:], func=AF.Exp,
                                             bias=ab[:, w * T + s:w * T + s + 1])
                        if s == 0:
                            nc.vector.tensor_copy(out=den[:], in_=e[:])
                            nc.vector.tensor_mul(out=acc[:], in0=e[:],
                                                 in1=hs[:, C:2 * C])
                        else:
                            nc.vector.tensor_add(out=den[:], in0=den[:], in1=e[:])
                            nc.vector.tensor_mul(
                                out=tmp[:], in0=e[:],
                                in1=hs[:, (s + 1) * C:(s + 2) * C])
                            nc.vector.tensor_add(out=acc[:], in0=acc[:],
                                                 in1=tmp[:])
                    rcp = ap_.tile([128, C], dt.float32, tag="rcp")
                    nc.vector.reciprocal(out=rcp[:], in_=den[:])
                    emb = ap_.tile([128, C], dt.float32, tag=f"emb{w}")
                    nc.vector.tensor_mul(out=emb[:], in0=acc[:], in1=rcp[:])
                    embs.append(emb)
                # --- weekly attention over W=3 ---
                wden = ap_.tile([128, C], dt.float32, tag="wden")
                wacc = ap_.tile([128, C], dt.float32, tag="wacc")
                wtmp = ap_.tile([128, C], dt.float32, tag="wtmp")
                for vv in range(W):
                    we = ep.tile([128, C], dt.float32, tag="we")
                    nc.vector.tensor_scalar_mul(
                        out=we[:], in0=embs[0][:],
                        scalar1=vw[:, vv * W:vv * W + 1])
                    for u in range(1, W):
                        nc.vector.scalar_tensor_tensor(
                            out=we[:], in0=embs[u][:],
                            scalar=vw[:, vv * W + u:vv * W + u + 1],
                            in1=we[:], op0=OP.mult, op1=OP.add)
                    nc.scalar.activation(out=we[:], in_=we[:], func=AF.Exp,
                                         bias=vb[:, vv:vv + 1])
                    if vv == 0:
                        nc.vector.tensor_copy(out=wden[:], in_=we[:])
                        nc.vector.tensor_mul(out=wacc[:], in0=we[:],
                                             in1=embs[0][:])
                    else:
                        nc.vector.tensor_add(out=wden[:], in0=wden[:], in1=we[:])
                        nc.vector.tensor_mul(out=wtmp[:], in0=we[:],
                                             in1=embs[vv][:])
                        nc.vector.tensor_add(out=wacc[:], in0=wacc[:],
                                             in1=wtmp[:])
                wrcp = ap_.tile([128, C], dt.float32, tag="wrcp")
                nc.vector.reciprocal(out=wrcp[:], in_=wden[:])
                wk = ap_.tile([128, C], dt.float32, tag="wk")
                nc.vector.tensor_mul(out=wk[:], in0=wacc[:], in1=wrcp[:])
                nc.sync.dma_start(out=wk_out[p], in_=wk[:])
    nc.compile()
    return nc


def _prep_weights(gru_wih, gru_whh, gru_bih, gru_bhh, att_w, att_b, ww_w, ww_b):
    # lhsT per (week, gate, parity): [K=128, M=64]; odd parity swaps halves
    wl = np.zeros((128, W * 4 * 2 * 64), np.float32)
    bl = np.zeros((128, W * 4), np.float32)
    for w in range(W):
        wih, whh = gru_wih[w], gru_whh[w]        # [3H, D], [3H, H]
        bih, bhh = gru_bih[w], gru_bhh[w]
        zero = np.zeros((64, 64), np.float32)
        for g, (xw, hw, bias) in enumerate([
                (wih[0:64], whh[0:64], bih[0:64] + bhh[0:64]),          # r
                (wih[64:128], whh[64:128], bih[64:128] + bhh[64:128]),  # z
                (zero, whh[128:192], bhh[128:192]),                     # hn
                (wih[128:192], zero, bih[128:192]),                     # xn
        ]):
            ce = ((w * 4 + g) * 2) * 64
            co = ce + 64
            wl[0:64, ce:ce + 64] = xw.T
            wl[64:128, ce:ce + 64] = hw.T
            wl[0:64, co:co + 64] = hw.T
            wl[64:128, co:co + 64] = xw.T
            bl[0:64, w * 4 + g] = bias
            bl[64:128, w * 4 + g] = bias
    aw = np.ascontiguousarray(np.broadcast_to(
        att_w.reshape(1, W * T * T), (128, W * T * T)), dtype=np.float32)
    ab = np.ascontiguousarray(np.broadcast_to(
        att_b.reshape(1, W * T), (128, W * T)), dtype=np.float32)
    vw = np.ascontiguousarray(np.broadcast_to(
        ww_w.reshape(1, W * W), (128, W * W)), dtype=np.float32)
    vb = np.ascontiguousarray(np.broadcast_to(
        ww_b.reshape(1, W), (128, W)), dtype=np.float32)
    return wl, bl, aw, ab, vw, vb


def kernel(x0, x1, x2, gru_wih, gru_whh, gru_bih, gru_bhh, att_w, att_b,
           ww_w, ww_b, gat_w, gat_att_src, gat_att_dst, gat_b,
           fus_w, fus_b, reg_w, reg_b, cls_w, cls_b, edge_index):
    if "nc" not in _cache:
        _cache["nc"] = _build_program()
    nc = _cache["nc"]
    wl, bl, aw, ab, vw, vb = _prep_weights(
        gru_wih, gru_whh, gru_bih, gru_bhh, att_w, att_b, ww_w, ww_b)
    # x in natural layout, bf16, interleaved [core, week]
    xg = np.zeros((NCORES * W, PER_CORE, T * D), BF16)
    for c in range(NCORES):
        lo = c * PER_CORE
        hi = min(N, lo + PER_CORE)
        if lo >= N:
            continue
        for w, xw in enumerate((x0, x1, x2)):
            xg[c * W + w, 0:hi - lo] = xw[lo:hi].reshape(hi - lo, T * D)
    in_maps = [{"x": xg[c * W:(c + 1) * W], "wl": wl, "bl": bl,
                "aw": aw, "ab": ab, "vw": vw, "vb": vb}
               for c in range(NCORES)]
    res = run_bass_kernel_spmd(nc, in_maps, list(range(NCORES)))
    _cache["exec_ns"] = res.exec_time_ns
    # weekly: [8, NPAIR, 128, C]; per core the 6 half-chunks map to node
    # blocks 0..4 (the 6th is the zero pad of the solo pair)
    wk = np.stack([res.results[c]["wk"] for c in range(NCORES)])
    weekly = wk.reshape(NCORES, NPAIR, 2, H, C).transpose(0, 1, 2, 4, 3) \
               .reshape(NCORES, 6, C, H)[:, :5].reshape(NPAD, H)[:N]

    # --- GAT on host (sorted-edge segment softmax + CSR aggregate) ---
    xgat = weekly @ gat_w.T
    asrc = xgat @ gat_att_src
    adst = xgat @ gat_att_dst
    loops = np.arange(N, dtype=edge_index.dtype)
    src = np.concatenate([edge_index[0], loops])
    dst = np.concatenate([edge_index[1], loops])
    perm = np.argsort(dst)
    ds, ss = dst[perm], src[perm]
    counts = np.bincount(ds, minlength=N)
    indptr = np.zeros(N + 1, np.int64)
    np.cumsum(counts, out=indptr[1:])
    starts = indptr[:-1]
    alpha = asrc[ss] + adst[ds]
    alpha = np.where(alpha > 0, alpha, 0.2 * alpha)
    amax = np.maximum.reduceat(alpha, starts)
    ex = np.exp(alpha - np.repeat(amax, counts))
    den = np.add.reduceat(ex, starts)
    coef = (ex / np.repeat(den, counts)).astype(np.float32)
    cat = csr_matrix((coef, ss, indptr), shape=(N, N)) @ xgat + gat_b

    fus = np.concatenate([weekly, cat], axis=-1) @ fus_w.T + fus_b
    fus = np.maximum(fus, 0.0)
    reg = np.ravel(fus @ reg_w.T + reg_b)
    cls = np.ravel(1.0 / (1.0 + np.exp(-(fus @ cls_w.T + cls_b))))
    return (reg.astype(np.float32), cls.astype(np.float32))


# revision 9
# speedup vs baseline: 7.2271x; 1.0237x over previous
import sys
sys.path.insert(0, "/opt/trn_rl_repo")
import numpy as np
import ml_dtypes
import concourse.bass as bass
import concourse.bacc as bacc
import concourse.mybir as mybir
import concourse.tile as tile
from concourse.bass_utils import run_bass_kernel_spmd
from scipy.sparse import csr_matrix

# Problem constants (hardcoded per contract)
N = 20000
T = 20
D = 64
H = 64
W = 3
NCORES = 8
NPAD = 20480            # 8 * 2560
PER_CORE = 2560         # padded per-core stocks (5 chunks of 512)
C = 512                 # chunk size
NPAIR = 3               # chunk pairs per core: (0,1), (2,3), (4, zero)
PAIRS = ((0, 1), (2, 3), (4, None))
dt = mybir.dt
BF16 = ml_dtypes.bfloat16

_cache = {}


def _build_program():
    """Bass program: per-week GRU + T-attention, then weekly attention.

    x arrives bf16 in natural node-major layout [W, PER_CORE, T*D] and is
    transposed on-device with XBAR DMA transposes of [512, 128] blocks: a
    block covers a (t=2k, t=2k+1) pair (contiguous 128 cols), transposing
    to [128, 512] with even-t features on partitions 0:64, odd-t on 64:128.

    GRU slot layout alternates by parity so no partition-shift is needed
    for x: even slot t has [x_t; h_{t-1}], odd slot t has [h_{t-1}; x_t];
    lhsT weights come in even/odd row-swapped variants.

    Attention (softmax over T without max-subtraction — |e| <= ~11 so exp
    is safe in f32) and the weekly W=3 attention run on-device; only the
    weekly embedding [NPAIR, 128, C] goes back to DRAM.
    """
    nc = bacc.Bacc("TRN2", target_bir_lowering=False, debug=False,
                   num_devices=NCORES)
    SLOTS = 21 * C
    x_in = nc.declare_dram_parameter("x", [W, PER_CORE, T * D], dt.bfloat16,
                                     isOutput=False)
    wl_in = nc.declare_dram_parameter("wl", [128, W * 4 * 2 * 64], dt.float32,
                                      isOutput=False)
    bl_in = nc.declare_dram_parameter("bl", [128, W * 4], dt.float32,
                                      isOutput=False)
    aw_in = nc.declare_dram_parameter("aw", [128, W * T * T], dt.float32,
                                      isOutput=False)
    ab_in = nc.declare_dram_parameter("ab", [128, W * T], dt.float32,
                                      isOutput=False)
    vw_in = nc.declare_dram_parameter("vw", [128, W * W], dt.float32,
                                      isOutput=False)
    vb_in = nc.declare_dram_parameter("vb", [128, W], dt.float32,
                                      isOutput=False)
    wk_out = nc.declare_dram_parameter("wk", [NPAIR, 128, C], dt.bfloat16,
                                       isOutput=True)
    AF = mybir.ActivationFunctionType
    OP = mybir.AluOpType

    with tile.TileContext(nc) as tc:
        with tc.tile_pool(name="wpool", bufs=1) as wpool, \
             tc.tile_pool(name="xbfp", bufs=2) as xbfp, \
             tc.tile_pool(name="xh", bufs=1) as xhp, \
             tc.tile_pool(name="hsp", bufs=1) as hsp, \
             tc.tile_pool(name="gate", bufs=2) as gp, \
             tc.tile_pool(name="attn", bufs=1) as ap_, \
             tc.tile_pool(name="ep", bufs=2) as ep, \
             tc.tile_pool(name="psum", bufs=2, space="PSUM") as pp:
            wl = wpool.tile([128, W * 4 * 2 * 64], dt.float32)
            bl = wpool.tile([128, W * 4], dt.float32)
            aw = wpool.tile([128, W * T * T], dt.float32)
            ab = wpool.tile([128, W * T], dt.float32)
            vw = wpool.tile([128, W * W], dt.float32)
            vb = wpool.tile([128, W], dt.float32)
            nc.sync.dma_start(out=wl[:], in_=wl_in[:, :])
            nc.sync.dma_start(out=bl[:], in_=bl_in[:, :])
            nc.sync.dma_start(out=aw[:], in_=aw_in[:, :])
            nc.sync.dma_start(out=ab[:], in_=ab_in[:, :])
            nc.sync.dma_start(out=vw[:], in_=vw_in[:, :])
            nc.sync.dma_start(out=vb[:], in_=vb_in[:, :])

            for p, (ca, cb) in enumerate(PAIRS):
                embs = []
                for w in range(W):
                    xh_A = xhp.tile([128, SLOTS], dt.float32, tag="xha")
                    xh_B = xhp.tile([128, SLOTS], dt.float32, tag="xhb")
                    hs = hsp.tile([128, SLOTS], dt.float32, tag="hs")
                    # --- load x via XBAR transpose-DMA, upcast to f32 ---
                    for half, (ck, tgt) in enumerate(((ca, xh_A), (cb, xh_B))):
                        if ck is None:
                            for i in range(T + 1):
                                nc.vector.memset(tgt[:, i * C:(i + 1) * C], 0.0)
                            continue
                        base = ck * C
                        for tb in range(T // 2):
                            xbf = xbfp.tile([128, C], dt.bfloat16,
                                            tag=f"xbf{half}")
                            nc.sync.dma_start(
                                out=xbf[:],
                                in_=x_in[w, base:base + C,
                                         tb * 128:(tb + 1) * 128],
                                transpose=True)
                            te, to = 2 * tb, 2 * tb + 1
                            nc.scalar.copy(
                                out=tgt[0:64, te * C:(te + 1) * C],
                                in_=xbf[0:64, :])
                            nc.scalar.copy(
                                out=tgt[64:128, to * C:(to + 1) * C],
                                in_=xbf[64:128, :])
                    # h_{-1} = 0 (slot 0 is even: h on rows 64:128)
                    nc.vector.memset(xh_A[64:128, 0:C], 0.0)
                    if cb is not None:
                        nc.vector.memset(xh_B[64:128, 0:C], 0.0)
                    nc.vector.memset(hs[:, 0:C], 0.0)
                    # --- GRU recurrence ---
                    for t in range(T):
                        par = t % 2
                        sl = slice(t * C, (t + 1) * C)
                        sl1 = slice((t + 1) * C, (t + 2) * C)
                        ps = []
                        for g in range(4):
                            pg = pp.tile([128, C], dt.float32, tag=f"g{g}")
                            col = ((w * 4 + g) * 2 + par) * 64
                            lt = wl[:, col:col + 64]
                            nc.tensor.matmul(out=pg[0:64, :], lhsT=lt,
                                             rhs=xh_A[:, sl],
                                             start=True, stop=True)
                            nc.tensor.matmul(out=pg[64:128, :], lhsT=lt,
                                             rhs=xh_B[:, sl],
                                             start=True, stop=True,
                                             tile_position=(0, 64))
                            ps.append(pg)
                        r = gp.tile([128, C], dt.float32, tag="r")
                        z = gp.tile([128, C], dt.float32, tag="z")
                        v = gp.tile([128, C], dt.float32, tag="v")
                        wo = gp.tile([128, C], dt.float32, tag="wo")
                        c_ = gp.tile([128, C], dt.float32, tag="c")
                        s_ = gp.tile([128, C], dt.float32, tag="s")
                        t_ = gp.tile([128, C], dt.float32, tag="t")
                        nc.scalar.activation(out=r[:], in_=ps[0][:],
                                             func=AF.Sigmoid,
                                             bias=bl[:, (w * 4):(w * 4 + 1)])
                        nc.scalar.activation(out=z[:], in_=ps[1][:],
                                             func=AF.Sigmoid,
                                             bias=bl[:, (w * 4 + 1):(w * 4 + 2)])
                        # v = (hn + b_hn) * r ; wo = (xn + b_in) + v
                        nc.vector.scalar_tensor_tensor(
                            out=v[:], in0=ps[2][:],
                            scalar=bl[:, (w * 4 + 2):(w * 4 + 3)],
                            in1=r[:], op0=OP.add, op1=OP.mult)
                        nc.vector.scalar_tensor_tensor(
                            out=wo[:], in0=ps[3][:],
                            scalar=bl[:, (w * 4 + 3):(w * 4 + 4)],
                            in1=v[:], op0=OP.add, op1=OP.add)
                        nc.scalar.activation(out=c_[:], in_=wo[:], func=AF.Tanh)
                        nc.vector.tensor_sub(out=s_[:], in0=hs[:, sl], in1=c_[:])
                        nc.vector.tensor_mul(out=t_[:], in0=z[:], in1=s_[:])
                        nc.vector.tensor_add(out=hs[:, sl1], in0=c_[:], in1=t_[:])
                        if t < T - 1:
                            if (t + 1) % 2 == 1:
                                # odd slot: h on rows 0:64
                                nc.sync.dma_start(out=xh_A[0:64, sl1],
                                                  in_=hs[0:64, sl1])
                                nc.sync.dma_start(out=xh_B[0:64, sl1],
                                                  in_=hs[64:128, sl1])
                            else:
                                # even slot: h on rows 64:128
                                nc.sync.dma_start(out=xh_A[64:128, sl1],
                                                  in_=hs[0:64, sl1])
                                nc.sync.dma_start(out=xh_B[64:128, sl1],
                                                  in_=hs[64:128, sl1])
                    # --- attention over T (softmax over s, streamed) ---
                    den = ap_.tile([128, C], dt.float32, tag="den")
                    acc = ap_.tile([128, C], dt.float32, tag="acc")
                    tmp = ap_.tile([128, C], dt.float32, tag="tmp")
                    for s in range(T):
                        e = ep.tile([128, C], dt.float32, tag="e")
                        a0 = w * T * T + s * T
                        nc.vector.tensor_scalar_mul(
                            out=e[:], in0=hs[:, C:2 * C],
                            scalar1=aw[:, a0:a0 + 1])
                        for t in range(1, T):
                            nc.vector.scalar_tensor_tensor(
                                out=e[:], in0=hs[:, (t + 1) * C:(t + 2) * C],
                                scalar=aw[:, a0 + t:a0 + t + 1],
                                in1=e[:], op0=OP.mult, op1=OP.add)
                        nc.scalar.activation(out=e[:], in_=e[:], func=AF.Exp,
                                             bias=ab[:, w * T + s:w * T + s + 1])
                        if s == 0:
                            nc.vector.tensor_copy(out=den[:], in_=e[:])
                            nc.vector.tensor_mul(out=acc[:], in0=e[:],
                                                 in1=hs[:, C:2 * C])
                        else:
                            nc.vector.tensor_add(out=den[:], in0=den[:], in1=e[:])
                            nc.vector.tensor_mul(
                                out=tmp[:], in0=e[:],
                                in1=hs[:, (s + 1) * C:(s + 2) * C])
                            nc.vector.tensor_add(out=acc[:], in0=acc[:],
                                                 in1=tmp[:])
                    rcp = ap_.tile([128, C], dt.float32, tag="rcp")
                    nc.vector.reciprocal(out=rcp[:], in_=den[:])
                    emb = ap_.tile([128, C], dt.float32, tag=f"emb{w}")
                    nc.vector.tensor_mul(out=emb[:], in0=acc[:], in1=rcp[:])
                    embs.append(emb)
                # --- weekly attention over W=3 ---
                wden = ap_.tile([128, C], dt.float32, tag="wden")
                wacc = ap_.tile([128, C], dt.float32, tag="wacc")
                wtmp = ap_.tile([128, C], dt.float32, tag="wtmp")
                for vv in range(W):
                    we = ep.tile([128, C], dt.float32, tag="we")
                    nc.vector.tensor_scalar_mul(
                        out=we[:], in0=embs[0][:],
                        scalar1=vw[:, vv * W:vv * W + 1])
                    for u in range(1, W):
                        nc.vector.scalar_tensor_tensor(
                            out=we[:], in0=embs[u][:],
                            scalar=vw[:, vv * W + u:vv * W + u + 1],
                            in1=we[:], op0=OP.mult, op1=OP.add)
                    nc.scalar.activation(out=we[:], in_=we[:], func=AF.Exp,
                                         bias=vb[:, vv:vv + 1])
                    if vv == 0:
                        nc.vector.tensor_copy(out=wden[:], in_=we[:])
                        nc.vector.tensor_mul(out=wacc[:], in0=we[:],
                                             in1=embs[0][:])
                    else:
                        nc.vector.tensor_add(out=wden[:], in0=wden[:], in1=we[:])
                        nc.vector.tensor_mul(out=wtmp[:], in0=we[:],
                                             in1=embs[vv][:])
                        nc.vector.tensor_add(out=wacc[:], in0=wacc[:],
                                             in1=wtmp[:])
                wrcp = ap_.tile([128, C], dt.float32, tag="wrcp")
                nc.vector.reciprocal(out=wrcp[:], in_=wden[:])
                wk = ap_.tile([128, C], dt.bfloat16, tag="wk")
                nc.vector.tensor_mul(out=wk[:], in0=wacc[:], in1=wrcp[:])
                nc.sync.dma_start(out=wk_out[p], in_=wk[:])
    nc.compile()
    return nc


def _prep_weights(gru_wih, gru_whh, gru_bih, gru_bhh, att_w, att_b, ww_w, ww_b):
    # lhsT per (week, gate, parity): [K=128, M=64]; odd parity swaps halves
    wl = np.zeros((128, W * 4 * 2 * 64), np.float32)
    bl = np.zeros((128, W * 4), np.float32)
    for w in range(W):
        wih, whh = gru_wih[w], gru_whh[w]        # [3H, D], [3H, H]
        bih, bhh = gru_bih[w], gru_bhh[w]
        zero = np.zeros((64, 64), np.float32)
        for g, (xw, hw, bias) in enumerate([
                (wih[0:64], whh[0:64], bih[0:64] + bhh[0:64]),          # r
                (wih[64:128], whh[64:128], bih[64:128] + bhh[64:128]),  # z
                (zero, whh[128:192], bhh[128:192]),                     # hn
                (wih[128:192], zero, bih[128:192]),                     # xn
        ]):
            ce = ((w * 4 + g) * 2) * 64
            co = ce + 64
            wl[0:64, ce:ce + 64] = xw.T
            wl[64:128, ce:ce + 64] = hw.T
            wl[0:64, co:co + 64] = hw.T
            wl[64:128, co:co + 64] = xw.T
            bl[0:64, w * 4 + g] = bias
            bl[64:128, w * 4 + g] = bias
    aw = np.ascontiguousarray(np.broadcast_to(
        att_w.reshape(1, W * T * T), (128, W * T * T)), dtype=np.float32)
    ab = np.ascontiguousarray(np.broadcast_to(
        att_b.reshape(1, W * T), (128, W * T)), dtype=np.float32)
    vw = np.ascontiguousarray(np.broadcast_to(
        ww_w.reshape(1, W * W), (128, W * W)), dtype=np.float32)
    vb = np.ascontiguousarray(np.broadcast_to(
        ww_b.reshape(1, W), (128, W)), dtype=np.float32)
    return wl, bl, aw, ab, vw, vb


def kernel(x0, x1, x2, gru_wih, gru_whh, gru_bih, gru_bhh, att_w, att_b,
           ww_w, ww_b, gat_w, gat_att_src, gat_att_dst, gat_b,
           fus_w, fus_b, reg_w, reg_b, cls_w, cls_b, edge_index):
    if "nc" not in _cache:
        _cache["nc"] = _build_program()
    nc = _cache["nc"]
    wl, bl, aw, ab, vw, vb = _prep_weights(
        gru_wih, gru_whh, gru_bih, gru_bhh, att_w, att_b, ww_w, ww_b)
    # x in natural layout, bf16, interleaved [core, week]
    xg = np.zeros((NCORES * W, PER_CORE, T * D), BF16)
    for c in range(NCORES):
        lo = c * PER_CORE
        hi = min(N, lo + PER_CORE)
        if lo >= N:
            continue
        for w, xw in enumerate((x0, x1, x2)):
            xg[c * W + w, 0:hi - lo] = xw[lo:hi].reshape(hi - lo, T * D)
    in_maps = [{"x": xg[c * W:(c + 1) * W], "wl": wl, "bl": bl,
                "aw": aw, "ab": ab, "vw": vw, "vb": vb}
               for c in range(NCORES)]
    res = run_bass_kernel_spmd(nc, in_maps, list(range(NCORES)))
    _cache["exec_ns"] = res.exec_time_ns
    # weekly: [8, NPAIR, 128, C]; per core the 6 half-chunks map to node
    # blocks 0..4 (the 6th is the zero pad of the solo pair)
    wk = np.stack([res.results[c]["wk"] for c in range(NCORES)]).astype(np.float32)
    weekly = wk.reshape(NCORES, NPAIR, 2, H, C).transpose(0, 1, 2, 4, 3) \
               .reshape(NCORES, 6, C, H)[:, :5].reshape(NPAD, H)[:N]

    # --- GAT on host (sorted-edge segment softmax + CSR aggregate) ---
    xgat = weekly @ gat_w.T
    asrc = xgat @ gat_att_src
    adst = xgat @ gat_att_dst
    loops = np.arange(N, dtype=edge_index.dtype)
    src = np.concatenate([edge_index[0], loops])
    dst = np.concatenate([edge_index[1], loops])
    perm = np.argsort(dst)
    ds, ss = dst[perm], src[perm]
    counts = np.bincount(ds, minlength=N)
    indptr = np.zeros(N + 1, np.int64)
    np.cumsum(counts, out=indptr[1:])
    starts = indptr[:-1]
    alpha = asrc[ss] + adst[ds]
    alpha = np.where(alpha > 0, alpha, 0.2 * alpha)
    amax = np.maximum.reduceat(alpha, starts)
    ex = np.exp(alpha - np.repeat(amax, counts))
    den = np.add.reduceat(ex, starts)
    coef = (ex / np.repeat(den, counts)).astype(np.float32)
    cat = csr_matrix((coef, ss, indptr), shape=(N, N)) @ xgat + gat_b

    fus = np.concatenate([weekly, cat], axis=-1) @ fus_w.T + fus_b
    fus = np.maximum(fus, 0.0)
    reg = np.ravel(fus @ reg_w.T + reg_b)
    cls = np.ravel(1.0 / (1.0 + np.exp(-(fus @ cls_w.T + cls_b))))
    return (reg.astype(np.float32), cls.astype(np.float32))


# revision 12
# speedup vs baseline: 9.5464x; 1.3209x over previous
import sys
sys.path.insert(0, "/opt/trn_rl_repo")
import numpy as np
import ml_dtypes
import concourse.bass as bass
import concourse.bacc as bacc
import concourse.mybir as mybir
import concourse.tile as tile
from scipy.sparse import csr_matrix

# Problem constants (hardcoded per contract)
N = 20000
T = 20
D = 64
H = 64
W = 3
NCORES = 8
NPAD = 20480            # 8 * 2560
PER_CORE = 2560         # padded per-core stocks (5 chunks of 512)
C = 512                 # chunk size
NPAIR = 3               # chunk pairs per core: (0,1), (2,3), (4, zero)
PAIRS = ((0, 1), (2, 3), (4, None))
dt = mybir.dt
BF16 = ml_dtypes.bfloat16

_cache = {}


def _build_program():
    """Bass program: per-week GRU + T-attention, then weekly attention.

    x arrives bf16 in natural node-major layout [W, PER_CORE, T*D] and is
    transposed on-device with XBAR DMA transposes of [512, 128] blocks: a
    block covers a (t=2k, t=2k+1) pair (contiguous 128 cols), transposing
    to [128, 512] with even-t features on partitions 0:64, odd-t on 64:128.

    GRU slot layout alternates by parity so no partition-shift is needed
    for x: even slot t has [x_t; h_{t-1}], odd slot t has [h_{t-1}; x_t];
    lhsT weights come in even/odd row-swapped variants.

    Attention (softmax over T without max-subtraction — |e| <= ~11 so exp
    is safe in f32) and the weekly W=3 attention run on-device; only the
    weekly embedding [NPAIR, 128, C] goes back to DRAM.
    """
    nc = bacc.Bacc("TRN2", target_bir_lowering=False, debug=False,
                   num_devices=NCORES)
    SLOTS = 21 * C
    x_in = nc.declare_dram_parameter("x", [W, PER_CORE, T * D], dt.bfloat16,
                                     isOutput=False)
    wl_in = nc.declare_dram_parameter("wl", [128, W * 4 * 2 * 64], dt.float32,
                                      isOutput=False)
    bl_in = nc.declare_dram_parameter("bl", [128, W * 4], dt.float32,
                                      isOutput=False)
    aw_in = nc.declare_dram_parameter("aw", [128, W * T * T], dt.float32,
                                      isOutput=False)
    ab_in = nc.declare_dram_parameter("ab", [128, W * T], dt.float32,
                                      isOutput=False)
    vw_in = nc.declare_dram_parameter("vw", [128, W * W], dt.float32,
                                      isOutput=False)
    vb_in = nc.declare_dram_parameter("vb", [128, W], dt.float32,
                                      isOutput=False)
    wk_out = nc.declare_dram_parameter("wk", [NPAIR, 128, C], dt.bfloat16,
                                       isOutput=True)
    AF = mybir.ActivationFunctionType
    OP = mybir.AluOpType

    with tile.TileContext(nc) as tc:
        with tc.tile_pool(name="wpool", bufs=1) as wpool, \
             tc.tile_pool(name="xbfp", bufs=2) as xbfp, \
             tc.tile_pool(name="xh", bufs=1) as xhp, \
             tc.tile_pool(name="hsp", bufs=1) as hsp, \
             tc.tile_pool(name="gate", bufs=2) as gp, \
             tc.tile_pool(name="attn", bufs=1) as ap_, \
             tc.tile_pool(name="ep", bufs=2) as ep, \
             tc.tile_pool(name="psum", bufs=2, space="PSUM") as pp:
            wl = wpool.tile([128, W * 4 * 2 * 64], dt.float32)
            bl = wpool.tile([128, W * 4], dt.float32)
            aw = wpool.tile([128, W * T * T], dt.float32)
            ab = wpool.tile([128, W * T], dt.float32)
            vw = wpool.tile([128, W * W], dt.float32)
            vb = wpool.tile([128, W], dt.float32)
            nc.sync.dma_start(out=wl[:], in_=wl_in[:, :])
            nc.sync.dma_start(out=bl[:], in_=bl_in[:, :])
            nc.sync.dma_start(out=aw[:], in_=aw_in[:, :])
            nc.sync.dma_start(out=ab[:], in_=ab_in[:, :])
            nc.sync.dma_start(out=vw[:], in_=vw_in[:, :])
            nc.sync.dma_start(out=vb[:], in_=vb_in[:, :])

            for p, (ca, cb) in enumerate(PAIRS):
                embs = []
                for w in range(W):
                    xh_A = xhp.tile([128, SLOTS], dt.float32, tag="xha")
                    xh_B = xhp.tile([128, SLOTS], dt.float32, tag="xhb")
                    hs = hsp.tile([128, SLOTS], dt.float32, tag="hs")
                    # --- load x via XBAR transpose-DMA, upcast to f32 ---
                    for half, (ck, tgt) in enumerate(((ca, xh_A), (cb, xh_B))):
                        if ck is None:
                            for i in range(T + 1):
                                nc.vector.memset(tgt[:, i * C:(i + 1) * C], 0.0)
                            continue
                        base = ck * C
                        for tb in range(T // 2):
                            xbf = xbfp.tile([128, C], dt.bfloat16,
                                            tag=f"xbf{half}")
                            nc.sync.dma_start(
                                out=xbf[:],
                                in_=x_in[w, base:base + C,
                                         tb * 128:(tb + 1) * 128],
                                transpose=True)
                            te, to = 2 * tb, 2 * tb + 1
                            nc.scalar.copy(
                                out=tgt[0:64, te * C:(te + 1) * C],
                                in_=xbf[0:64, :])
                            nc.scalar.copy(
                                out=tgt[64:128, to * C:(to + 1) * C],
                                in_=xbf[64:128, :])
                    # h_{-1} = 0 (slot 0 is even: h on rows 64:128)
                    nc.vector.memset(xh_A[64:128, 0:C], 0.0)
                    if cb is not None:
                        nc.vector.memset(xh_B[64:128, 0:C], 0.0)
                    nc.vector.memset(hs[:, 0:C], 0.0)
                    # --- GRU recurrence ---
                    for t in range(T):
                        par = t % 2
                        sl = slice(t * C, (t + 1) * C)
                        sl1 = slice((t + 1) * C, (t + 2) * C)
                        ps = []
                        for g in range(4):
                            pg = pp.tile([128, C], dt.float32, tag=f"g{g}")
                            col = ((w * 4 + g) * 2 + par) * 64
                            lt = wl[:, col:col + 64]
                            nc.tensor.matmul(out=pg[0:64, :], lhsT=lt,
                                             rhs=xh_A[:, sl],
                                             start=True, stop=True)
                            nc.tensor.matmul(out=pg[64:128, :], lhsT=lt,
                                             rhs=xh_B[:, sl],
                                             start=True, stop=True,
                                             tile_position=(0, 64))
                            ps.append(pg)
                        r = gp.tile([128, C], dt.float32, tag="r")
                        z = gp.tile([128, C], dt.float32, tag="z")
                        v = gp.tile([128, C], dt.float32, tag="v")
                        wo = gp.tile([128, C], dt.float32, tag="wo")
                        c_ = gp.tile([128, C], dt.float32, tag="c")
                        s_ = gp.tile([128, C], dt.float32, tag="s")
                        t_ = gp.tile([128, C], dt.float32, tag="t")
                        nc.scalar.activation(out=r[:], in_=ps[0][:],
                                             func=AF.Sigmoid,
                                             bias=bl[:, (w * 4):(w * 4 + 1)])
                        nc.scalar.activation(out=z[:], in_=ps[1][:],
                                             func=AF.Sigmoid,
                                             bias=bl[:, (w * 4 + 1):(w * 4 + 2)])
                        # v = (hn + b_hn) * r ; wo = (xn + b_in) + v
                        nc.vector.scalar_tensor_tensor(
                            out=v[:], in0=ps[2][:],
                            scalar=bl[:, (w * 4 + 2):(w * 4 + 3)],
                            in1=r[:], op0=OP.add, op1=OP.mult)
                        nc.vector.scalar_tensor_tensor(
                            out=wo[:], in0=ps[3][:],
                            scalar=bl[:, (w * 4 + 3):(w * 4 + 4)],
                            in1=v[:], op0=OP.add, op1=OP.add)
                        nc.scalar.activation(out=c_[:], in_=wo[:], func=AF.Tanh)
                        nc.vector.tensor_sub(out=s_[:], in0=hs[:, sl], in1=c_[:])
                        nc.vector.tensor_mul(out=t_[:], in0=z[:], in1=s_[:])
                        nc.vector.tensor_add(out=hs[:, sl1], in0=c_[:], in1=t_[:])
                        if t < T - 1:
                            if (t + 1) % 2 == 1:
                                # odd slot: h on rows 0:64
                                nc.sync.dma_start(out=xh_A[0:64, sl1],
                                                  in_=hs[0:64, sl1])
                                nc.sync.dma_start(out=xh_B[0:64, sl1],
                                                  in_=hs[64:128, sl1])
                            else:
                                # even slot: h on rows 64:128
                                nc.sync.dma_start(out=xh_A[64:128, sl1],
                                                  in_=hs[0:64, sl1])
                                nc.sync.dma_start(out=xh_B[64:128, sl1],
                                                  in_=hs[64:128, sl1])
                    # --- attention over T (softmax over s, streamed) ---
                    den = ap_.tile([128, C], dt.float32, tag="den")
                    acc = ap_.tile([128, C], dt.float32, tag="acc")
                    tmp = ap_.tile([128, C], dt.float32, tag="tmp")
                    for s in range(T):
                        e = ep.tile([128, C], dt.float32, tag="e")
                        a0 = w * T * T + s * T
                        nc.vector.tensor_scalar_mul(
                            out=e[:], in0=hs[:, C:2 * C],
                            scalar1=aw[:, a0:a0 + 1])
                        for t in range(1, T):
                            nc.vector.scalar_tensor_tensor(
                                out=e[:], in0=hs[:, (t + 1) * C:(t + 2) * C],
                                scalar=aw[:, a0 + t:a0 + t + 1],
                                in1=e[:], op0=OP.mult, op1=OP.add)
                        nc.scalar.activation(out=e[:], in_=e[:], func=AF.Exp,
                                             bias=ab[:, w * T + s:w * T + s + 1])
                        if s == 0:
                            nc.vector.tensor_copy(out=den[:], in_=e[:])
                            nc.vector.tensor_mul(out=acc[:], in0=e[:],
                                                 in1=hs[:, C:2 * C])
                        else:
                            nc.vector.tensor_add(out=den[:], in0=den[:], in1=e[:])
                            nc.vector.tensor_mul(
                                out=tmp[:], in0=e[:],
                                in1=hs[:, (s + 1) * C:(s + 2) * C])
                            nc.vector.tensor_add(out=acc[:], in0=acc[:],
                                                 in1=tmp[:])
                    rcp = ap_.tile([128, C], dt.float32, tag="rcp")
                    nc.vector.reciprocal(out=rcp[:], in_=den[:])
                    emb = ap_.tile([128, C], dt.float32, tag=f"emb{w}")
                    nc.vector.tensor_mul(out=emb[:], in0=acc[:], in1=rcp[:])
                    embs.append(emb)
                # --- weekly attention over W=3 ---
                wden = ap_.tile([128, C], dt.float32, tag="wden")
                wacc = ap_.tile([128, C], dt.float32, tag="wacc")
                wtmp = ap_.tile([128, C], dt.float32, tag="wtmp")
                for vv in range(W):
                    we = ep.tile([128, C], dt.float32, tag="we")
                    nc.vector.tensor_scalar_mul(
                        out=we[:], in0=embs[0][:],
                        scalar1=vw[:, vv * W:vv * W + 1])
                    for u in range(1, W):
                        nc.vector.scalar_tensor_tensor(
                            out=we[:], in0=embs[u][:],
                            scalar=vw[:, vv * W + u:vv * W + u + 1],
                            in1=we[:], op0=OP.mult, op1=OP.add)
                    nc.scalar.activation(out=we[:], in_=we[:], func=AF.Exp,
                                         bias=vb[:, vv:vv + 1])
                    if vv == 0:
                        nc.vector.tensor_copy(out=wden[:], in_=we[:])
                        nc.vector.tensor_mul(out=wacc[:], in0=we[:],
                                             in1=embs[0][:])
                    else:
                        nc.vector.tensor_add(out=wden[:], in0=wden[:], in1=we[:])
                        nc.vector.tensor_mul(out=wtmp[:], in0=we[:],
                                             in1=embs[vv][:])
                        nc.vector.tensor_add(out=wacc[:], in0=wacc[:],
                                             in1=wtmp[:])
                wrcp = ap_.tile([128, C], dt.float32, tag="wrcp")
                nc.vector.reciprocal(out=wrcp[:], in_=wden[:])
                wk = ap_.tile([128, C], dt.bfloat16, tag="wk")
                nc.vector.tensor_mul(out=wk[:], in0=wacc[:], in1=wrcp[:])
                nc.sync.dma_start(out=wk_out[p], in_=wk[:])
    nc.compile()
    return nc


def _get_dispatch(nc):
    """SPMD dispatch over the 8 cores — the axon path of
    bass_utils.run_bass_kernel_spmd (bass2jax custom call + shard_map over
    jax.devices()), but with the jitted callable cached across calls and a
    single host conversion per output instead of one per core."""
    import jax
    from jax.sharding import Mesh, PartitionSpec
    from jax.experimental.shard_map import shard_map
    from concourse.bass2jax import (_bass_exec_p, partition_id_tensor,
                                    install_neuronx_cc_hook)
    install_neuronx_cc_hook()
    partition_name = (nc.partition_id_tensor.name
                      if nc.partition_id_tensor else None)
    in_names, out_names, out_avals, out_shapes = [], [], [], []
    for alloc in nc.m.functions[0].allocations:
        if not isinstance(alloc, mybir.MemoryLocationSet):
            continue
        name = alloc.memorylocations[0].name
        if alloc.kind == "ExternalInput":
            if name != partition_name:
                in_names.append(name)
        elif alloc.kind == "ExternalOutput":
            shape = tuple(alloc.tensor_shape)
            dtype = mybir.dt.np(alloc.dtype)
            out_avals.append(jax.core.ShapedArray(shape, dtype))
            out_names.append(name)
            out_shapes.append((shape, dtype))
    n_params = len(in_names)
    n_outs = len(out_avals)
    in_names_all = tuple(in_names + out_names
                         + ([partition_name] if partition_name else []))

    def _body(*args):
        operands = list(args)
        if partition_name is not None:
            operands.append(partition_id_tensor())
        return tuple(_bass_exec_p.bind(
            *operands, out_avals=tuple(out_avals), in_names=in_names_all,
            out_names=tuple(out_names), lowering_input_output_aliases=(),
            sim_require_finite=True, sim_require_nnan=True, nc=nc))

    devices = jax.devices()[:NCORES]
    mesh = Mesh(np.asarray(devices), ("core",))
    donate = tuple(range(n_params, n_params + n_outs))
    sharded = jax.jit(
        shard_map(_body, mesh=mesh,
                  in_specs=(PartitionSpec("core"),) * (n_params + n_outs),
                  out_specs=(PartitionSpec("core"),) * n_outs,
                  check_rep=False),
        donate_argnums=donate, keep_unused=True)
    return sharded, in_names, out_names, out_shapes


def _run(nc, global_ins):
    """Run the program SPMD on cores 0..7; inputs/outputs are global arrays
    with axis 0 = NCORES * per-core dim."""
    if "dispatch" not in _cache:
        _cache["dispatch"] = _get_dispatch(nc)
    sharded, in_names, out_names, out_shapes = _cache["dispatch"]
    ins = [global_ins[nm] for nm in in_names]
    zeros = [np.zeros((NCORES * s[0], *s[1:]), d) for s, d in out_shapes]
    outs = sharded(*ins, *zeros)
    return {nm: np.asarray(outs[i]) for i, nm in enumerate(out_names)}


def _prep_weights(gru_wih, gru_whh, gru_bih, gru_bhh, att_w, att_b, ww_w, ww_b):
    # lhsT per (week, gate, parity): [K=128, M=64]; odd parity swaps halves
    wl = np.zeros((128, W * 4 * 2 * 64), np.float32)
    bl = np.zeros((128, W * 4), np.float32)
    for w in range(W):
        wih, whh = gru_wih[w], gru_whh[w]        # [3H, D], [3H, H]
        bih, bhh = gru_bih[w], gru_bhh[w]
        zero = np.zeros((64, 64), np.float32)
        for g, (xw, hw, bias) in enumerate([
                (wih[0:64], whh[0:64], bih[0:64] + bhh[0:64]),          # r
                (wih[64:128], whh[64:128], bih[64:128] + bhh[64:128]),  # z
                (zero, whh[128:192], bhh[128:192]),                     # hn
                (wih[128:192], zero, bih[128:192]),                     # xn
        ]):
            ce = ((w * 4 + g) * 2) * 64
            co = ce + 64
            wl[0:64, ce:ce + 64] = xw.T
            wl[64:128, ce:ce + 64] = hw.T
            wl[0:64, co:co + 64] = hw.T
            wl[64:128, co:co + 64] = xw.T
            bl[0:64, w * 4 + g] = bias
            bl[64:128, w * 4 + g] = bias
    aw = np.ascontiguousarray(np.broadcast_to(
        att_w.reshape(1, W * T * T), (128, W * T * T)), dtype=np.float32)
    ab = np.ascontiguousarray(np.broadcast_to(
        att_b.reshape(1, W * T), (128, W * T)), dtype=np.float32)
    vw = np.ascontiguousarray(np.broadcast_to(
        ww_w.reshape(1, W * W), (128, W * W)), dtype=np.float32)
    vb = np.ascontiguousarray(np.broadcast_to(
        ww_b.reshape(1, W), (128, W)), dtype=np.float32)
    return wl, bl, aw, ab, vw, vb


def kernel(x0, x1, x2, gru_wih, gru_whh, gru_bih, gru_bhh, att_w, att_b,
           ww_w, ww_b, gat_w, gat_att_src, gat_att_dst, gat_b,
           fus_w, fus_b, reg_w, reg_b, cls_w, cls_b, edge_index):
    if "nc" not in _cache:
        _cache["nc"] = _build_program()
    nc = _cache["nc"]
    wl, bl, aw, ab, vw, vb = _prep_weights(
        gru_wih, gru_whh, gru_bih, gru_bhh, att_w, att_b, ww_w, ww_b)
    # x in natural layout, bf16, interleaved [core, week]
    xg = np.zeros((NCORES * W, PER_CORE, T * D), BF16)
    for c in range(NCORES):
        lo = c * PER_CORE
        hi = min(N, lo + PER_CORE)
        if lo >= N:
            continue
        for w, xw in enumerate((x0, x1, x2)):
            xg[c * W + w, 0:hi - lo] = xw[lo:hi].reshape(hi - lo, T * D)
    gl = {"x": xg, "wl": np.tile(wl, (NCORES, 1)),
          "bl": np.tile(bl, (NCORES, 1)), "aw": np.tile(aw, (NCORES, 1)),
          "ab": np.tile(ab, (NCORES, 1)), "vw": np.tile(vw, (NCORES, 1)),
          "vb": np.tile(vb, (NCORES, 1))}
    res = _run(nc, gl)
    _cache["exec_ns"] = None
    # weekly: [8, NPAIR, 128, C]; per core the 6 half-chunks map to node
    # blocks 0..4 (the 6th is the zero pad of the solo pair)
    wk = res["wk"].reshape(NCORES, NPAIR, 128, C).astype(np.float32)
    weekly = wk.reshape(NCORES, NPAIR, 2, H, C).transpose(0, 1, 2, 4, 3) \
               .reshape(NCORES, 6, C, H)[:, :5].reshape(NPAD, H)[:N]

    # --- GAT on host (sorted-edge segment softmax + CSR aggregate) ---
    xgat = weekly @ gat_w.T
    asrc = xgat @ gat_att_src
    adst = xgat @ gat_att_dst
    loops = np.arange(N, dtype=edge_index.dtype)
    src = np.concatenate([edge_index[0], loops])
    dst = np.concatenate([edge_index[1], loops])
    perm = np.argsort(dst)
    ds, ss = dst[perm], src[perm]
    counts = np.bincount(ds, minlength=N)
    indptr = np.zeros(N + 1, np.int64)
    np.cumsum(counts, out=indptr[1:])
    starts = indptr[:-1]
    alpha = asrc[ss] + adst[ds]
    alpha = np.where(alpha > 0, alpha, 0.2 * alpha)
    amax = np.maximum.reduceat(alpha, starts)
    ex = np.exp(alpha - np.repeat(amax, counts))
    den = np.add.reduceat(ex, starts)
    coef = (ex / np.repeat(den, counts)).astype(np.float32)
    cat = csr_matrix((coef, ss, indptr), shape=(N, N)) @ xgat + gat_b

    fus = np.concatenate([weekly, cat], axis=-1) @ fus_w.T + fus_b
    fus = np.maximum(fus, 0.0)
    reg = np.ravel(fus @ reg_w.T + reg_b)
    cls = np.ravel(1.0 / (1.0 + np.exp(-(fus @ cls_w.T + cls_b))))
    return (reg.astype(np.float32), cls.astype(np.float32))
